# revision 13
# baseline (speedup 1.0000x reference)
"""NeRF render kernel for 8 TRN2 NeuronCores (pure data parallel over rays).

Per core: 512 rays x 64 samples, MLP width 256 x 8 layers + rgb/density heads,
then alpha-composite. Layout: activations [features(partition), rays(free)],
one sample-tile = 1 sample x 512 rays.

Fast path (samples 0-62): hidden layers run as fp8-e4m3 DoubleRow matmuls
(K=256 contracted per instruction); weights e4m3, activations quantized to
e4m3 directly by one merged [128,1024] relu per layer (all biases in this
problem are structurally zero). L0/heads fp8. The Fourier encoder is an fp32
matmul (2 samples packed per instruction) emitting pos*2^k/2pi + cos-phase,
magic-number rounding extracts n, and 2pi*(frac) + phase-bias feeds the HW
Sin. Sample 63 (tau = density*1e10 makes the density relu sign knife-edge)
runs in f32r end-to-end with a 2-step 2pi range reduction.

Schedule: tiles advance in 3-tile lockstep units (2 DR matmuls per tile per
layer on PE while the other tiles' relus run on Act/DVE, alternating), with
group prep (jpos on GpSimd, reduction, sin) and next-unit L0s emitted across
unit boundaries. PSUM: 4 rotating [128,1024] bank-pairs; heads write into the
tile's layer-7 pair; head outputs scatter via an SBUF stage + 2 DMAs/tile.
"""
import os
import numpy as np
import ml_dtypes

NB = 10
ENC = 60
WIDTH = 256
S = 64
RPC = 512  # rays per core
N_CORES = 8
NEAR, FAR = 0.1, 4.0
MAGIC = float(1.5 * 2**23)
INV2PI = float(1.0 / (2 * np.pi))
TWO_PI_F32 = float(np.float32(2 * np.pi))
P2HI = 6.28125  # 2pi hi word, exact in 8 mantissa bits
P2LO = float(2 * np.pi - 6.28125)

E4 = ml_dtypes.float8_e4m3
E5 = ml_dtypes.float8_e5m2

LAST_EXEC_NS = None
_CACHE = {}


def _build_nc():
    import concourse.bacc as bacc
    import concourse.tile as tile
    from concourse import mybir

    dt = mybir.dt
    AF = mybir.ActivationFunctionType
    ALU = mybir.AluOpType
    PM = mybir.MatmulPerfMode
    f32 = dt.float32
    f8h = dt.float8e4
    f8l = dt.float8e5
    f32r = dt.float32r

    nc = bacc.Bacc("TRN2", target_bir_lowering=False, debug=False,
                   num_devices=N_CORES)

    def din(name, shape, dtype=f32):
        return nc.dram_tensor(name, shape, dtype, kind="ExternalInput")

    HB = 7 * 2 * 128  # hidden DR weight block columns
    d_jit = din("jitter_t", [S, RPC])
    d_rp = din("ray_pos_t", [3, RPC])
    d_rd = din("ray_dir_t", [3, RPC])
    d_win8 = din("win8", [124, WIDTH], f8h)
    d_win_32 = din("win_32", [ENC, WIDTH], f32r)
    d_whid_hi = din("whid_hi", [128, 2, HB], f8h)
    d_whid_32 = din("whid_32", [128, 7 * 2 * WIDTH], f32r)
    d_whd_hi = din("whd_hi", [128, 2, 16], f8h)
    d_whd_32 = din("whd_32", [128, 8], f32r)
    d_brep01 = din("brep01", [128, 124])
    d_brep23 = din("brep23", [128, 124])
    d_brepa01 = din("brepa01", [128, 124])
    d_brepa23 = din("brepa23", [128, 124])
    d_colc = din("colc", [124, 1])
    d_bca = din("bca", [128, 1])
    d_bcb = din("bcb", [S, 1])
    d_bcd = din("bcd", [S, 1])
    d_mbias = din("mbias", [124, 1])
    d_b2 = din("b2", [124, 1])
    d_iota = din("iota", [S, 1])
    d_tris = din("tris", [S, S])
    d_onesb = din("onesb", [128, 2])
    d_big = din("big", [1, RPC])
    d_out = nc.dram_tensor("out", [4, RPC], f32, kind="ExternalOutput")

    # merged relu (biases are structurally zero in this problem): one
    # [128,1024] op per layer, alternating Act/DVE by (layer+tile) parity
    def engines(nc):
        def act(out, in_):
            nc.scalar.activation(out, in_, AF.Relu)

        def dve(out, in_):
            nc.vector.tensor_scalar(out, in_, 0.0, None, ALU.max)

        return (act, dve)

    with tile.TileContext(nc) as tc:
        with (
            tc.tile_pool(name="static", bufs=1) as sp,
            tc.tile_pool(name="act", bufs=7) as ap,
            tc.tile_pool(name="misc", bufs=3) as mp,
            tc.tile_pool(name="red", bufs=2) as rp,
            tc.tile_pool(name="comp", bufs=1) as cp,
            tc.tile_pool(name="ps_l", bufs=4, space="PSUM") as pl,
        ):
            def load(dram, shape, dtype, tag, eng=None):
                t = sp.tile(shape, dtype, tag=tag)
                (eng or nc.sync).dma_start(t[:], dram[:])
                return t

            jt = load(d_jit, [S, RPC], f32, "jt")
            iota = load(d_iota, [S, 1], f32, "iota")
            brep01 = load(d_brep01, [128, 124], f32, "brep01")
            brep23 = load(d_brep23, [128, 124], f32, "brep23")
            colc = load(d_colc, [124, 1], f32, "colc")
            win8 = load(d_win8, [124, WIDTH], f8h, "win8")
            whid_hi = load(d_whid_hi, [128, 2, HB], f8h, "whid_hi")
            whd_hi = load(d_whd_hi, [128, 2, 16], f8h, "whd_hi")
            whid_32 = load(d_whid_32, [128, 7 * 2 * WIDTH], f32r, "whid_32",
                           nc.gpsimd)
            brepa01 = load(d_brepa01, [128, 124], f32, "brepa01", nc.gpsimd)
            brepa23 = load(d_brepa23, [128, 124], f32, "brepa23", nc.gpsimd)
            win_32 = load(d_win_32, [ENC, WIDTH], f32r, "win_32", nc.gpsimd)
            whd_32 = load(d_whd_32, [128, 8], f32r, "whd_32", nc.gpsimd)
            bca = load(d_bca, [128, 1], f32, "bca", nc.gpsimd)
            bcb = load(d_bcb, [S, 1], f32, "bcb", nc.gpsimd)
            bcd = load(d_bcd, [S, 1], f32, "bcd", nc.gpsimd)
            mbias = load(d_mbias, [124, 1], f32, "mbias", nc.gpsimd)
            b2 = load(d_b2, [124, 1], f32, "b2", nc.gpsimd)
            tris = load(d_tris, [S, S], f32, "tris", nc.gpsimd)
            onesb = load(d_onesb, [128, 2], f32, "onesb", nc.gpsimd)

            rp128 = sp.tile([128, RPC], f32, tag="rp128")
            rd128 = sp.tile([128, RPC], f32, tag="rd128")
            nc.vector.memset(rp128[:], 1.0)
            nc.vector.memset(rd128[:], 0.0)
            for j in range(4):
                nc.sync.dma_start(rp128[32 * j:32 * j + 3, :], d_rp[:, :])
                nc.sync.dma_start(rd128[32 * j:32 * j + 3, :], d_rd[:, :])

            # depths = 0.1 + (3.9 * (idx + jitter)) / 64, exact fp32 op order
            ddtmp = sp.tile([S, RPC], f32, tag="ddtmp")
            nc.vector.tensor_scalar(ddtmp[:], jt[:], iota[:], 3.9, ALU.add, ALU.mult)
            dd = sp.tile([S, RPC], f32, tag="dd")
            nc.vector.tensor_scalar(dd[:], ddtmp[:], float(1.0 / 64), 0.1, ALU.mult, ALU.add)

            ddsh = cp.tile([S, RPC], f32, tag="ddsh")
            nc.sync.dma_start(ddsh[0:63, :], dd[1:64, :])
            nc.sync.dma_start(ddsh[63:64, :], d_big[:])
            delt = cp.tile([S, RPC], f32, tag="delt")
            nc.vector.tensor_sub(delt[:], ddsh[:], dd[:])

            # composite accumulation buffers
            cmpA = cp.tile([128, RPC], f32, tag="cmpA")  # rgb0 (0-63), rgb1 (64-127)
            cmpB = cp.tile([128, RPC], f32, tag="cmpB")  # rgb2 (0-63), den (64-127)

            ENG = engines(nc)

            def prep_group(g):
                """jpos, enc matmuls, range reduction, sin for samples 4g..4g+3.

                Returns (enc8, enc32) -- enc8 [120,1024] e4m3; enc32 [60,512]
                f32 only for the last group (sample 63)."""
                s0 = 4 * g
                acc = (g == 15)
                dd4 = mp.tile([128, RPC], f32, tag="dd4")
                for i in range(3):
                    nc.sync.dma_start(dd4[i::32, :], dd[s0:s0 + 4, :])
                jeng = nc.vector if g < 2 else nc.gpsimd
                jtmp = mp.tile([128, RPC], f32, tag="jtmp")
                jeng.tensor_mul(jtmp[:], dd4[:], rd128[:])
                jpos = mp.tile([128, RPC], f32, tag="jpos")
                jeng.tensor_add(jpos[:], jtmp[:], rp128[:])

                pe = pl.tile([128, 1024], f32, tag="lp")
                b01, b23 = (brepa01, brepa23) if acc else (brep01, brep23)
                nc.tensor.matmul(pe[0:124, 0:512], b01[:], jpos[:],
                                 start=True, stop=True)
                nc.tensor.matmul(pe[0:124, 512:1024], b23[:], jpos[:],
                                 start=True, stop=True)
                xb = pe[0:124, :]
                enc8 = ap.tile([124, 1024], f8h, tag="enc8")
                if not acc:
                    # xb = pos*2^k/2pi + mbias; n = round(xb); r = 2pi*(xb - n)
                    # - 2pi*mbias + b2 folded into sin bias column
                    rn = rp.tile([124, 1024], f32, tag="red_n")
                    nc.vector.tensor_scalar(rn[:], xb, MAGIC, MAGIC,
                                            ALU.add, ALU.subtract)
                    ru = rp.tile([124, 1024], f32, tag="red_u")
                    nc.scalar.activation(ru[:], rn[:], AF.Copy, scale=-1.0)
                    rr = rp.tile([124, 1024], f32, tag="red_r")
                    nc.vector.tensor_tensor(rr[:], xb, ru[:], ALU.add)
                    nc.scalar.activation(enc8[:], rr[:], AF.Sin, bias=colc[:],
                                         scale=TWO_PI_F32)
                    return enc8, None
                # acc group (sample 63): original high-precision path
                rt = rp.tile([124, 1024], f32, tag="red_t")
                nc.vector.tensor_scalar(rt[:], xb, INV2PI, mbias[:], ALU.mult, ALU.add)
                rn = rp.tile([124, 1024], f32, tag="red_n")
                nc.vector.tensor_scalar(rn[:], rt[:], MAGIC, MAGIC, ALU.add, ALU.subtract)
                ru = rp.tile([124, 1024], f32, tag="red_u")
                nc.scalar.activation(ru[:], rn[:], AF.Copy, scale=-P2HI)
                rr = rp.tile([124, 1024], f32, tag="red_r")
                nc.vector.scalar_tensor_tensor(rr[:], ru[:], b2[:], xb, ALU.add, ALU.add)
                ru2 = rp.tile([124, 1024], f32, tag="red_u2")
                nc.vector.tensor_scalar(ru2[:], rn[:], P2LO, None, ALU.mult)
                nc.vector.tensor_sub(rr[:], rr[:], ru2[:])
                nc.scalar.activation(enc8[:], rr[:], AF.Sin)
                enc32 = mp.tile([ENC, RPC], f32r, tag="enc32")
                nc.scalar.activation(enc32[:], rr[64:124, 512:1024], AF.Sin)
                return enc8, enc32

            def enc_slice(enc8, s):
                r0 = 64 * (s & 1)
                c0 = 512 * ((s >> 1) & 1)
                return enc8[r0:r0 + 60, c0:c0 + 512]

            def mlp8_layer(l, x_in, x_out, todd):
                """One fp8 hidden layer: 2 DR matmuls + 1 merged relu."""
                p = pl.tile([128, 1024], f32, tag="lp")
                for m in range(2):
                    blk = ((l - 1) * 2 + m) * 128
                    nc.tensor.matmul(p[:, 512 * m:512 * m + 512],
                                     whid_hi[:, :, blk:blk + 128], x_in[:, :, :],
                                     start=True, stop=True, perf_mode=PM.DoubleRow)
                ENG[(l + todd) & 1](x_out[:, :, :], p[:, 0:1024])
                return p

            def mlp32_layer(l, x_in, x_out, todd):
                """One f32r hidden layer (sample 63 path)."""
                p = pl.tile([128, 1024], f32, tag="lp")
                for m in range(2):
                    for kc in range(2):
                        base = ((l - 1) * 2 + kc) * WIDTH + m * 128
                        nc.tensor.matmul(p[:, 512 * m:512 * m + 512],
                                         whid_32[:, base:base + 128],
                                         x_in[:, 512 * kc:512 * kc + 512],
                                         start=(kc == 0), stop=(kc == 1))
                ENG[(l + todd) & 1](x_out[:], p[:, 0:1024])
                return p

            def l0_stage8(enc8, s, i):
                es = enc_slice(enc8, s)
                r0 = 64 * (s & 1)
                p0 = pl.tile([128, 1024], f32, tag="lp")
                for m in range(2):
                    nc.tensor.matmul(p0[:, 512 * m:512 * m + 512],
                                     win8[r0:r0 + 60, 128 * m:128 * m + 128], es,
                                     start=True, stop=True)
                x = ap.tile([128, 2, 512], f8h, tag="x8")
                ENG[i & 1](x[:, :, :], p0[:, 0:1024])
                return x

            def layer_stage8(l, x, i):
                xn = ap.tile([128, 2, 512], f8h, tag="x8")
                p = mlp8_layer(l, x, xn, i)
                return xn if l < 7 else (xn, p)

            def head_stage8(x, p7, s):
                nc.tensor.matmul(p7[0:16, 0:512], whd_hi[:, :, :],
                                 x[:, :, :], start=True, stop=True,
                                 perf_mode=PM.DoubleRow)

            def l0_stage32(enc32, i):
                p0 = pl.tile([128, 1024], f32, tag="lp")
                nc.tensor.matmul(p0[:, 0:512], win_32[:, 0:128], enc32[:],
                                 start=True, stop=True)
                nc.tensor.matmul(p0[:, 512:1024], win_32[:, 128:256], enc32[:],
                                 start=True, stop=True)
                x = mp.tile([128, 1024], f32r, tag="x32")
                ENG[i & 1](x[:], p0[:, 0:1024])
                return x

            def layer_stage32(l, x, i):
                xn = mp.tile([128, 1024], f32r, tag="x32")
                p = mlp32_layer(l, x, xn, i)
                return xn if l < 7 else (xn, p)

            def head_stage32(x, p7, s):
                for kc in range(2):
                    nc.tensor.matmul(p7[0:4, 0:512],
                                     whd_32[:, kc * 4:kc * 4 + 4],
                                     x[:, 512 * kc:512 * kc + 512],
                                     start=(kc == 0), stop=(kc == 1))

            def scatter_tile(s, p7, todd):
                stg = mp.tile([16, RPC], f32, tag="stg")
                nc.scalar.copy(stg[:], p7[0:16, 0:512])
                nc.sync.dma_start(cmpA[s::64, :], stg[0:2, :])
                nc.sync.dma_start(cmpB[s::64, :], stg[2:4, :])

            def l0_unit(unit, preps):
                st = {}
                for i, s in enumerate(unit):
                    enc8, enc32 = preps[s >> 2]
                    if s == 63:
                        st[s] = (layer_stage32, head_stage32,
                                 l0_stage32(enc32, i))
                    else:
                        st[s] = (layer_stage8, head_stage8,
                                 l0_stage8(enc8, s, i))
                return st

            def body_unit(unit, st):
                for l in range(1, 8):
                    for i, s in enumerate(unit):
                        fl, fh, x = st[s]
                        st[s] = (fl, fh, fl(l, x, i))

            def finish_unit(unit, st):
                for s in unit:
                    fl, fh, (x, p7) = st[s]
                    fh(x, p7, s)
                    scatter_tile(s, p7, (s // 3) & 1)

            units = [tuple(range(t, t + 3)) for t in range(0, 60, 3)]
            units.append((60, 61, 62, 63))
            preps = {}
            next_prep = [0]

            def ensure_prep(upto):
                while next_prep[0] <= min(upto, 15):
                    g = next_prep[0]
                    preps[g] = prep_group(g)
                    next_prep[0] += 1

            ensure_prep(1)
            st = l0_unit(units[0], preps)
            for u, unit in enumerate(units):
                body_unit(unit, st)
                ensure_prep((max(unit) + 4) >> 2)
                if u + 1 < len(units):
                    st_next = l0_unit(units[u + 1], preps)
                else:
                    st_next = None
                finish_unit(unit, st)
                st = st_next

            # ---- head activations ----
            tmpa = cp.tile([128, RPC], f32, tag="tmpa")
            nc.scalar.activation(tmpa[:], cmpA[:], AF.Tanh, bias=bca[:], scale=0.5)
            nc.vector.tensor_scalar(cmpA[:], tmpa[:], 0.5, 0.5, ALU.mult, ALU.add)
            tmpb = cp.tile([S, RPC], f32, tag="tmpb")
            nc.scalar.activation(tmpb[:], cmpB[0:S, :], AF.Tanh, bias=bcb[:], scale=0.5)
            denr = cp.tile([S, RPC], f32, tag="denr")
            nc.vector.tensor_scalar(denr[:], cmpB[S:128, :], bcd[:], 0.0,
                                    ALU.add, ALU.max)
            nc.vector.tensor_scalar(cmpB[0:S, :], tmpb[:], 0.5, 0.5, ALU.mult, ALU.add)

            # ---- volume rendering composite ----
            tau = cp.tile([S, RPC], f32, tag="tau")
            nc.vector.tensor_mul(tau[:], denr[:], delt[:])
            pep = pl.tile([128, 1024], f32, tag="lp")
            nc.tensor.matmul(pep[0:S, 0:512], tris[:], tau[:], start=True, stop=True)
            inc = cp.tile([S, RPC], f32, tag="inc")
            nc.vector.tensor_add(inc[:], pep[0:S, 0:512], tau[:])
            exc2 = cp.tile([S, RPC], f32, tag="exc2")
            nc.vector.tensor_sub(exc2[:], inc[:], tau[:])
            trans = cp.tile([S, RPC], f32, tag="trans")
            nc.scalar.activation(trans[:], exc2[:], AF.Exp, scale=-1.0)
            ee = cp.tile([S, RPC], f32, tag="ee")
            nc.scalar.activation(ee[:], tau[:], AF.Exp, scale=-1.0)
            alpha = cp.tile([S, RPC], f32, tag="alpha")
            nc.vector.tensor_scalar(alpha[:], ee[:], -1.0, 1.0, ALU.mult, ALU.add)
            wt = cp.tile([S, RPC], f32, tag="wt")
            nc.vector.tensor_mul(wt[:], alpha[:], trans[:])
            w2 = cp.tile([128, RPC], f32, tag="w2")
            nc.sync.dma_start(w2[0:S, :], wt[:])
            nc.sync.dma_start(w2[S:128, :], wt[:])
            nc.sync.dma_start(cmpB[S:128, :], dd[:])
            wa = cp.tile([128, RPC], f32, tag="wa")
            nc.vector.tensor_mul(wa[:], w2[:], cmpA[:])
            wb = cp.tile([128, RPC], f32, tag="wb")
            nc.vector.tensor_mul(wb[:], w2[:], cmpB[:])
            nc.tensor.matmul(pep[0:2, 512:1024], onesb[:], wa[:], start=True,
                             stop=True, tile_position=(0, 0))
            nc.tensor.matmul(pep[32:34, 512:1024], onesb[:], wb[:], start=True,
                             stop=True, tile_position=(0, 32))
            outsb = cp.tile([S, RPC], f32, tag="outsb")
            nc.vector.tensor_copy(outsb[0:2, :], pep[0:2, 512:1024])
            nc.vector.tensor_copy(outsb[32:34, :], pep[32:34, 512:1024])
            nc.sync.dma_start(d_out[0:2, :], outsb[0:2, :])
            nc.sync.dma_start(d_out[2:4, :], outsb[32:34, :])

    nc.compile()
    return nc


def _prep(inputs):
    jt = np.ascontiguousarray(np.asarray(inputs["jitter"], np.float32).T)
    rpt = np.ascontiguousarray(np.asarray(inputs["ray_pos"], np.float32).T)
    rdt = np.ascontiguousarray(np.asarray(inputs["ray_dir"], np.float32).T)
    w_in = np.asarray(inputs["w_in"], np.float32)
    perm = np.empty(ENC, np.int64)
    for r in range(ENC):
        base = 0 if r < 30 else 10
        rr = r % 30
        perm[r] = (rr // 10) * 20 + base + (rr % 10)
    win_p = np.ascontiguousarray(w_in[perm])
    w8 = win_p.astype(E4)
    win8 = np.zeros((124, WIDTH), E4)
    win8[0:60] = w8
    win8[64:124] = w8

    w_hid = np.asarray(inputs["w_hid"], np.float32)
    HB = 7 * 2 * 128
    whid_hi = np.empty((128, 2, HB), E4)
    whid_cat = np.empty((128, 7 * 2 * WIDTH), np.float32)
    for l in range(7):
        W = w_hid[l]  # [256, 256]
        Wh = W.astype(E4)
        for m in range(2):
            blk = (l * 2 + m) * 128
            cols = slice(m * 128, (m + 1) * 128)
            whid_hi[:, 0, blk:blk + 128] = Wh[0:128, cols]
            whid_hi[:, 1, blk:blk + 128] = Wh[128:256, cols]
        for kc in range(2):
            whid_cat[:, (l * 2 + kc) * WIDTH:(l * 2 + kc + 1) * WIDTH] = \
                W[kc * 128:(kc + 1) * 128, :]

    whd = np.concatenate([np.asarray(inputs["w_rgb"], np.float32),
                          np.asarray(inputs["w_den"], np.float32)], axis=1)
    whd_h = whd.astype(E4)
    whd_hi = np.zeros((128, 2, 16), E4)
    whd_hi[:, 0, 0:4] = whd_h[0:128]
    whd_hi[:, 1, 0:4] = whd_h[128:256]
    whd_cat = np.empty((128, 8), np.float32)
    whd_cat[:, 0:4] = whd[0:128]
    whd_cat[:, 4:8] = whd[128:256]

    b_rgb = np.asarray(inputs["b_rgb"], np.float32)
    b_den = np.asarray(inputs["b_den"], np.float32)
    bca = np.zeros((128, 1), np.float32)
    bca[0:S] = 0.5 * b_rgb[0]
    bca[S:128] = 0.5 * b_rgb[1]
    bcb = np.full((S, 1), 0.5 * b_rgb[2], np.float32)
    bcd = np.full((S, 1), b_den[0], np.float32)

    # brep for 2-sample-packed fp32 encoder matmul: rows 32j+i of jpos map to
    # output rows 60*(j&1) + perm-row r with weight 2^k
    brepa01 = np.zeros((128, 124), np.float32)
    brepa23 = np.zeros((128, 124), np.float32)
    for r in range(ENC):
        rr = r % 30
        i, k = rr // 10, rr % 10
        brepa01[0 + i, r] = float(2.0 ** k)
        brepa01[32 + i, 64 + r] = float(2.0 ** k)
        brepa23[64 + i, r] = float(2.0 ** k)
        brepa23[96 + i, 64 + r] = float(2.0 ** k)
    mbias = np.zeros((124, 1), np.float32)
    mbias[30:60] = 0.25
    mbias[94:124] = 0.25
    b2v = np.zeros((124, 1), np.float32)
    b2v[30:60] = np.float32(np.pi / 2)
    b2v[94:124] = np.float32(np.pi / 2)
    # fast-path brep: rows scaled by 1/2pi, plus mbias via the ones-row 32j+3
    inv2pi = np.float64(1.0) / (2 * np.pi)
    brep01 = brepa01 * np.float32(inv2pi)
    brep23 = brepa23 * np.float32(inv2pi)
    for r in range(124):
        if mbias[r, 0]:
            if r < 64:
                brep01[3, r] = mbias[r, 0]
                brep23[67, r] = mbias[r, 0]
            else:
                brep01[35, r] = mbias[r, 0]
                brep23[99, r] = mbias[r, 0]
    # sin bias column: b2 - 2pi*mbias
    colc = (b2v - np.float32(2 * np.pi) * mbias).astype(np.float32)
    iota = np.arange(S, dtype=np.float32).reshape(S, 1)
    tris = (np.arange(S)[:, None] < np.arange(S)[None, :]).astype(np.float32)
    onesb = np.zeros((128, 2), np.float32)
    onesb[:64, 0] = 1.0
    onesb[64:, 1] = 1.0
    big = np.full((1, RPC), 1e10, np.float32)
    common = dict(win8=win8, win_32=win_p,
                  whid_hi=whid_hi, whid_32=whid_cat,
                  whd_hi=whd_hi, whd_32=whd_cat,
                  brep01=brep01, brep23=brep23,
                  brepa01=brepa01, brepa23=brepa23, colc=colc,
                  bca=bca, bcb=bcb, bcd=bcd, mbias=mbias, b2=b2v,
                  iota=iota, tris=tris, onesb=onesb, big=big)
    in_maps = []
    for c in range(N_CORES):
        sl = slice(c * RPC, (c + 1) * RPC)
        m = dict(common)
        m["jitter_t"] = np.ascontiguousarray(jt[:, sl])
        m["ray_pos_t"] = np.ascontiguousarray(rpt[:, sl])
        m["ray_dir_t"] = np.ascontiguousarray(rdt[:, sl])
        in_maps.append(m)
    return in_maps


def kernel(**inputs):
    global LAST_EXEC_NS
    from concourse.bass_utils import run_bass_kernel_spmd
    if "nc" not in _CACHE:
        _CACHE["nc"] = _build_nc()
    nc = _CACHE["nc"]
    in_maps = _prep(inputs)
    trace = bool(os.environ.get("KERNEL_TRACE"))
    res = run_bass_kernel_spmd(nc, in_maps, core_ids=list(range(N_CORES)),
                               trace=trace)
    LAST_EXEC_NS = getattr(res, "exec_time_ns", None)
    _CACHE["last_results"] = res.results
    _CACHE["last_res"] = res
    out = np.empty((N_CORES * RPC, 4), np.float32)
    for c in range(N_CORES):
        out[c * RPC:(c + 1) * RPC] = res.results[c]["out"].T
    return out


# revision 14
# speedup vs baseline: 1.0510x; 1.0510x over previous
"""NeRF render kernel for 8 TRN2 NeuronCores (pure data parallel over rays).

Per core: 512 rays x 64 samples, MLP width 256 x 8 layers + rgb/density heads,
then alpha-composite. Layout: activations [features(partition), rays(free)],
one sample-tile = 1 sample x 512 rays.

Fast path (samples 0-62): hidden layers run as fp8-e4m3 DoubleRow matmuls
(K=256 contracted per instruction); weights e4m3, activations quantized to
e4m3 directly by one merged [128,1024] relu per layer (all biases in this
problem are structurally zero). L0/heads fp8. The Fourier encoder is an fp32
matmul (2 samples packed per instruction) emitting pos*2^k/2pi + cos-phase,
magic-number rounding extracts n, and 2pi*(frac) + phase-bias feeds the HW
Sin. Sample 63 (tau = density*1e10 makes the density relu sign knife-edge)
runs in f32r end-to-end with a 2-step 2pi range reduction.

Schedule: tiles advance in 3-tile lockstep units (2 DR matmuls per tile per
layer on PE while the other tiles' relus run on Act/DVE, alternating), with
group prep (jpos on GpSimd, reduction, sin) and next-unit L0s emitted across
unit boundaries. PSUM: 4 rotating [128,1024] bank-pairs; heads write into the
tile's layer-7 pair; head outputs scatter via an SBUF stage + 2 DMAs/tile.
"""
import os
import numpy as np
import ml_dtypes

NB = 10
ENC = 60
WIDTH = 256
S = 64
RPC = 512  # rays per core
N_CORES = 8
NEAR, FAR = 0.1, 4.0
MAGIC = float(1.5 * 2**23)
INV2PI = float(1.0 / (2 * np.pi))
TWO_PI_F32 = float(np.float32(2 * np.pi))
P2HI = 6.28125  # 2pi hi word, exact in 8 mantissa bits
P2LO = float(2 * np.pi - 6.28125)

E4 = ml_dtypes.float8_e4m3
E5 = ml_dtypes.float8_e5m2

LAST_EXEC_NS = None
_CACHE = {}


def _build_nc():
    import concourse.bacc as bacc
    import concourse.tile as tile
    from concourse import mybir

    dt = mybir.dt
    AF = mybir.ActivationFunctionType
    ALU = mybir.AluOpType
    PM = mybir.MatmulPerfMode
    f32 = dt.float32
    f8h = dt.float8e4
    f8l = dt.float8e5
    f32r = dt.float32r

    nc = bacc.Bacc("TRN2", target_bir_lowering=False, debug=False,
                   num_devices=N_CORES)

    def din(name, shape, dtype=f32):
        return nc.dram_tensor(name, shape, dtype, kind="ExternalInput")

    HB = 7 * 2 * 128  # hidden DR weight block columns
    d_jit = din("jitter_t", [S, RPC])
    d_rp = din("ray_pos_t", [3, RPC])
    d_rd = din("ray_dir_t", [3, RPC])
    d_win8 = din("win8", [124, WIDTH], f8h)
    d_win_32 = din("win_32", [ENC, WIDTH], f32r)
    d_whid_hi = din("whid_hi", [128, 2, HB], f8h)
    d_whid_32 = din("whid_32", [128, 7 * 2 * WIDTH], f32r)
    d_whd_hi = din("whd_hi", [128, 2, 16], f8h)
    d_whd_32 = din("whd_32", [128, 8], f32r)
    d_brep01 = din("brep01", [128, 124])
    d_brep23 = din("brep23", [128, 124])
    d_brepa01 = din("brepa01", [128, 124])
    d_brepa23 = din("brepa23", [128, 124])
    d_colc = din("colc", [124, 1])
    d_bca = din("bca", [128, 1])
    d_bcb = din("bcb", [S, 1])
    d_bcd = din("bcd", [S, 1])
    d_mbias = din("mbias", [124, 1])
    d_b2 = din("b2", [124, 1])
    d_iota = din("iota", [S, 1])
    d_tris = din("tris", [S, S])
    d_onesb = din("onesb", [128, 2])
    d_big = din("big", [1, RPC])
    d_out = nc.dram_tensor("out", [4, RPC], f32, kind="ExternalOutput")

    # merged relu (biases are structurally zero in this problem): one
    # [128,1024] op per layer, alternating Act/DVE by (layer+tile) parity
    def engines(nc):
        def act(out, in_):
            nc.scalar.activation(out, in_, AF.Relu)

        def dve(out, in_):
            nc.vector.tensor_scalar(out, in_, 0.0, None, ALU.max)

        return (act, dve)

    with tile.TileContext(nc) as tc:
        with (
            tc.tile_pool(name="static", bufs=1) as sp,
            tc.tile_pool(name="act", bufs=7) as ap,
            tc.tile_pool(name="misc", bufs=3) as mp,
            tc.tile_pool(name="red", bufs=2) as rp,
            tc.tile_pool(name="comp", bufs=1) as cp,
            tc.tile_pool(name="ps_l", bufs=4, space="PSUM") as pl,
        ):
            def load(dram, shape, dtype, tag, eng=None):
                t = sp.tile(shape, dtype, tag=tag)
                (eng or nc.sync).dma_start(t[:], dram[:])
                return t

            jt = load(d_jit, [S, RPC], f32, "jt")
            iota = load(d_iota, [S, 1], f32, "iota")
            brep01 = load(d_brep01, [128, 124], f32, "brep01")
            brep23 = load(d_brep23, [128, 124], f32, "brep23")
            colc = load(d_colc, [124, 1], f32, "colc")
            win8 = load(d_win8, [124, WIDTH], f8h, "win8")
            whid_hi = load(d_whid_hi, [128, 2, HB], f8h, "whid_hi")
            whd_hi = load(d_whd_hi, [128, 2, 16], f8h, "whd_hi")
            whid_32 = load(d_whid_32, [128, 7 * 2 * WIDTH], f32r, "whid_32",
                           nc.gpsimd)
            brepa01 = load(d_brepa01, [128, 124], f32, "brepa01", nc.gpsimd)
            brepa23 = load(d_brepa23, [128, 124], f32, "brepa23", nc.gpsimd)
            win_32 = load(d_win_32, [ENC, WIDTH], f32r, "win_32", nc.gpsimd)
            whd_32 = load(d_whd_32, [128, 8], f32r, "whd_32", nc.gpsimd)
            bca = load(d_bca, [128, 1], f32, "bca", nc.gpsimd)
            bcb = load(d_bcb, [S, 1], f32, "bcb", nc.gpsimd)
            bcd = load(d_bcd, [S, 1], f32, "bcd", nc.gpsimd)
            mbias = load(d_mbias, [124, 1], f32, "mbias", nc.gpsimd)
            b2 = load(d_b2, [124, 1], f32, "b2", nc.gpsimd)
            tris = load(d_tris, [S, S], f32, "tris", nc.gpsimd)
            onesb = load(d_onesb, [128, 2], f32, "onesb", nc.gpsimd)

            rp128 = sp.tile([128, RPC], f32, tag="rp128")
            rd128 = sp.tile([128, RPC], f32, tag="rd128")
            nc.vector.memset(rp128[:], 1.0)
            nc.vector.memset(rd128[:], 0.0)
            for j in range(4):
                nc.sync.dma_start(rp128[32 * j:32 * j + 3, :], d_rp[:, :])
                nc.sync.dma_start(rd128[32 * j:32 * j + 3, :], d_rd[:, :])

            # depths = 0.1 + (3.9 * (idx + jitter)) / 64, exact fp32 op order
            ddtmp = sp.tile([S, RPC], f32, tag="ddtmp")
            nc.vector.tensor_scalar(ddtmp[:], jt[:], iota[:], 3.9, ALU.add, ALU.mult)
            dd = sp.tile([S, RPC], f32, tag="dd")
            nc.vector.tensor_scalar(dd[:], ddtmp[:], float(1.0 / 64), 0.1, ALU.mult, ALU.add)

            ddsh = cp.tile([S, RPC], f32, tag="ddsh")
            nc.sync.dma_start(ddsh[0:63, :], dd[1:64, :])
            nc.sync.dma_start(ddsh[63:64, :], d_big[:])
            delt = cp.tile([S, RPC], f32, tag="delt")
            nc.vector.tensor_sub(delt[:], ddsh[:], dd[:])

            # composite accumulation buffers
            cmpA = cp.tile([128, RPC], f32, tag="cmpA")  # rgb0 (0-63), rgb1 (64-127)
            cmpB = cp.tile([128, RPC], f32, tag="cmpB")  # rgb2 (0-63), den (64-127)

            ENG = engines(nc)

            def prep_group(g):
                """jpos, enc matmuls, range reduction, sin for samples 4g..4g+3.

                Returns (enc8, enc32) -- enc8 [120,1024] e4m3; enc32 [60,512]
                f32 only for the last group (sample 63)."""
                s0 = 4 * g
                acc = (g == 15)
                dd4 = mp.tile([128, RPC], f32, tag="dd4")
                for i in range(3):
                    nc.sync.dma_start(dd4[i::32, :], dd[s0:s0 + 4, :])
                jeng = nc.vector if g < 2 else nc.gpsimd
                jtmp = mp.tile([128, RPC], f32, tag="jtmp")
                jeng.tensor_mul(jtmp[:], dd4[:], rd128[:])
                jpos = mp.tile([128, RPC], f32, tag="jpos")
                jeng.tensor_add(jpos[:], jtmp[:], rp128[:])

                pe = pl.tile([128, 1024], f32, tag="lp")
                b01, b23 = (brepa01, brepa23) if acc else (brep01, brep23)
                nc.tensor.matmul(pe[0:124, 0:512], b01[:], jpos[:],
                                 start=True, stop=True)
                nc.tensor.matmul(pe[0:124, 512:1024], b23[:], jpos[:],
                                 start=True, stop=True)
                xb = pe[0:124, :]
                enc8 = ap.tile([124, 1024], f8h, tag="enc8")
                if not acc:
                    # xb = pos*2^k/2pi + mbias; n = round(xb); r = 2pi*(xb - n)
                    # - 2pi*mbias + b2 folded into sin bias column
                    rn = rp.tile([124, 1024], f32, tag="red_n")
                    nc.vector.tensor_scalar(rn[:], xb, MAGIC, MAGIC,
                                            ALU.add, ALU.subtract)
                    ru = rp.tile([124, 1024], f32, tag="red_u")
                    nc.scalar.activation(ru[:], rn[:], AF.Copy, scale=-1.0)
                    rr = rp.tile([124, 1024], f32, tag="red_r")
                    nc.vector.tensor_tensor(rr[:], xb, ru[:], ALU.add)
                    nc.scalar.activation(enc8[:], rr[:], AF.Sin, bias=colc[:],
                                         scale=TWO_PI_F32)
                    return enc8, None
                # acc group (sample 63): original high-precision path
                rt = rp.tile([124, 1024], f32, tag="red_t")
                nc.vector.tensor_scalar(rt[:], xb, INV2PI, mbias[:], ALU.mult, ALU.add)
                rn = rp.tile([124, 1024], f32, tag="red_n")
                nc.vector.tensor_scalar(rn[:], rt[:], MAGIC, MAGIC, ALU.add, ALU.subtract)
                ru = rp.tile([124, 1024], f32, tag="red_u")
                nc.scalar.activation(ru[:], rn[:], AF.Copy, scale=-P2HI)
                rr = rp.tile([124, 1024], f32, tag="red_r")
                nc.vector.scalar_tensor_tensor(rr[:], ru[:], b2[:], xb, ALU.add, ALU.add)
                ru2 = rp.tile([124, 1024], f32, tag="red_u2")
                nc.vector.tensor_scalar(ru2[:], rn[:], P2LO, None, ALU.mult)
                nc.vector.tensor_sub(rr[:], rr[:], ru2[:])
                nc.scalar.activation(enc8[:], rr[:], AF.Sin)
                enc32 = mp.tile([ENC, RPC], f32r, tag="enc32")
                nc.scalar.activation(enc32[:], rr[64:124, 512:1024], AF.Sin)
                return enc8, enc32

            def enc_slice(enc8, s):
                r0 = 64 * (s & 1)
                c0 = 512 * ((s >> 1) & 1)
                return enc8[r0:r0 + 60, c0:c0 + 512]

            def mlp8_layer(l, x_in, x_out, todd):
                """One fp8 hidden layer: 2 DR matmuls + 1 merged relu."""
                p = pl.tile([128, 1024], f32, tag="lp")
                for m in range(2):
                    blk = ((l - 1) * 2 + m) * 128
                    nc.tensor.matmul(p[:, 512 * m:512 * m + 512],
                                     whid_hi[:, :, blk:blk + 128], x_in[:, :, :],
                                     start=True, stop=True, perf_mode=PM.DoubleRow)
                ENG[(l + todd) & 1](x_out[:, :, :], p[:, 0:1024])
                return p

            def mlp32_layer(l, x_in, x_out, todd):
                """One f32r hidden layer (sample 63 path)."""
                p = pl.tile([128, 1024], f32, tag="lp")
                for m in range(2):
                    for kc in range(2):
                        base = ((l - 1) * 2 + kc) * WIDTH + m * 128
                        nc.tensor.matmul(p[:, 512 * m:512 * m + 512],
                                         whid_32[:, base:base + 128],
                                         x_in[:, 512 * kc:512 * kc + 512],
                                         start=(kc == 0), stop=(kc == 1))
                ENG[(l + todd) & 1](x_out[:], p[:, 0:1024])
                return p

            def l0_stage8(enc8, s, i):
                es = enc_slice(enc8, s)
                r0 = 64 * (s & 1)
                p0 = pl.tile([128, 1024], f32, tag="lp")
                for m in range(2):
                    nc.tensor.matmul(p0[:, 512 * m:512 * m + 512],
                                     win8[r0:r0 + 60, 128 * m:128 * m + 128], es,
                                     start=True, stop=True)
                x = ap.tile([128, 2, 512], f8h, tag="x8")
                ENG[i & 1](x[:, :, :], p0[:, 0:1024])
                return x

            def layer_stage8(l, x, i):
                xn = ap.tile([128, 2, 512], f8h, tag="x8")
                p = mlp8_layer(l, x, xn, i)
                return xn if l < 7 else (xn, p)

            def head_stage8(x, p7, s):
                nc.tensor.matmul(p7[0:16, 0:512], whd_hi[:, :, :],
                                 x[:, :, :], start=True, stop=True,
                                 perf_mode=PM.DoubleRow)

            def l0_stage32(enc32, i):
                p0 = pl.tile([128, 1024], f32, tag="lp")
                nc.tensor.matmul(p0[:, 0:512], win_32[:, 0:128], enc32[:],
                                 start=True, stop=True)
                nc.tensor.matmul(p0[:, 512:1024], win_32[:, 128:256], enc32[:],
                                 start=True, stop=True)
                x = mp.tile([128, 1024], f32r, tag="x32")
                ENG[i & 1](x[:], p0[:, 0:1024])
                return x

            def layer_stage32(l, x, i):
                xn = mp.tile([128, 1024], f32r, tag="x32")
                p = mlp32_layer(l, x, xn, i)
                return xn if l < 7 else (xn, p)

            def head_stage32(x, p7, s):
                for kc in range(2):
                    nc.tensor.matmul(p7[0:4, 0:512],
                                     whd_32[:, kc * 4:kc * 4 + 4],
                                     x[:, 512 * kc:512 * kc + 512],
                                     start=(kc == 0), stop=(kc == 1))

            def scatter_tile(s, p7, todd):
                stg = mp.tile([16, RPC], f32, tag="stg")
                nc.scalar.copy(stg[:], p7[0:16, 0:512])
                nc.sync.dma_start(cmpA[s::64, :], stg[0:2, :])
                nc.sync.dma_start(cmpB[s::64, :], stg[2:4, :])

            def l0_unit(unit, preps):
                st = {}
                for i, s in enumerate(unit):
                    enc8, enc32 = preps[s >> 2]
                    if s == 63:
                        st[s] = (layer_stage32, head_stage32,
                                 l0_stage32(enc32, i))
                    else:
                        st[s] = (layer_stage8, head_stage8,
                                 l0_stage8(enc8, s, i))
                return st

            def body_unit(unit, st):
                for l in range(1, 8):
                    for i, s in enumerate(unit):
                        fl, fh, x = st[s]
                        st[s] = (fl, fh, fl(l, x, i))

            def finish_unit(unit, st):
                for s in unit:
                    fl, fh, (x, p7) = st[s]
                    fh(x, p7, s)
                    scatter_tile(s, p7, (s // 3) & 1)

            units = [tuple(range(t, t + 3)) for t in range(0, 60, 3)]
            units.append((60, 61, 62, 63))
            preps = {}
            next_prep = [0]

            def ensure_prep(upto):
                while next_prep[0] <= min(upto, 15):
                    g = next_prep[0]
                    preps[g] = prep_group(g)
                    next_prep[0] += 1

            ensure_prep(1)
            st = l0_unit(units[0], preps)
            for u, unit in enumerate(units):
                body_unit(unit, st)
                ensure_prep((max(unit) + 8) >> 2)
                if u + 1 < len(units):
                    st_next = l0_unit(units[u + 1], preps)
                else:
                    st_next = None
                finish_unit(unit, st)
                st = st_next

            # ---- head activations ----
            tmpa = cp.tile([128, RPC], f32, tag="tmpa")
            nc.scalar.activation(tmpa[:], cmpA[:], AF.Tanh, bias=bca[:], scale=0.5)
            nc.vector.tensor_scalar(cmpA[:], tmpa[:], 0.5, 0.5, ALU.mult, ALU.add)
            tmpb = cp.tile([S, RPC], f32, tag="tmpb")
            nc.scalar.activation(tmpb[:], cmpB[0:S, :], AF.Tanh, bias=bcb[:], scale=0.5)
            denr = cp.tile([S, RPC], f32, tag="denr")
            nc.vector.tensor_scalar(denr[:], cmpB[S:128, :], bcd[:], 0.0,
                                    ALU.add, ALU.max)
            nc.vector.tensor_scalar(cmpB[0:S, :], tmpb[:], 0.5, 0.5, ALU.mult, ALU.add)

            # ---- volume rendering composite ----
            tau = cp.tile([S, RPC], f32, tag="tau")
            nc.vector.tensor_mul(tau[:], denr[:], delt[:])
            pep = pl.tile([128, 1024], f32, tag="lp")
            nc.tensor.matmul(pep[0:S, 0:512], tris[:], tau[:], start=True, stop=True)
            inc = cp.tile([S, RPC], f32, tag="inc")
            nc.vector.tensor_add(inc[:], pep[0:S, 0:512], tau[:])
            exc2 = cp.tile([S, RPC], f32, tag="exc2")
            nc.vector.tensor_sub(exc2[:], inc[:], tau[:])
            trans = cp.tile([S, RPC], f32, tag="trans")
            nc.scalar.activation(trans[:], exc2[:], AF.Exp, scale=-1.0)
            ee = cp.tile([S, RPC], f32, tag="ee")
            nc.scalar.activation(ee[:], tau[:], AF.Exp, scale=-1.0)
            alpha = cp.tile([S, RPC], f32, tag="alpha")
            nc.vector.tensor_scalar(alpha[:], ee[:], -1.0, 1.0, ALU.mult, ALU.add)
            wt = cp.tile([S, RPC], f32, tag="wt")
            nc.vector.tensor_mul(wt[:], alpha[:], trans[:])
            w2 = cp.tile([128, RPC], f32, tag="w2")
            nc.sync.dma_start(w2[0:S, :], wt[:])
            nc.sync.dma_start(w2[S:128, :], wt[:])
            nc.sync.dma_start(cmpB[S:128, :], dd[:])
            wa = cp.tile([128, RPC], f32, tag="wa")
            nc.vector.tensor_mul(wa[:], w2[:], cmpA[:])
            wb = cp.tile([128, RPC], f32, tag="wb")
            nc.vector.tensor_mul(wb[:], w2[:], cmpB[:])
            nc.tensor.matmul(pep[0:2, 512:1024], onesb[:], wa[:], start=True,
                             stop=True, tile_position=(0, 0))
            nc.tensor.matmul(pep[32:34, 512:1024], onesb[:], wb[:], start=True,
                             stop=True, tile_position=(0, 32))
            outsb = cp.tile([S, RPC], f32, tag="outsb")
            nc.vector.tensor_copy(outsb[0:2, :], pep[0:2, 512:1024])
            nc.vector.tensor_copy(outsb[32:34, :], pep[32:34, 512:1024])
            nc.sync.dma_start(d_out[0:2, :], outsb[0:2, :])
            nc.sync.dma_start(d_out[2:4, :], outsb[32:34, :])

    nc.compile()
    return nc


def _prep(inputs):
    jt = np.ascontiguousarray(np.asarray(inputs["jitter"], np.float32).T)
    rpt = np.ascontiguousarray(np.asarray(inputs["ray_pos"], np.float32).T)
    rdt = np.ascontiguousarray(np.asarray(inputs["ray_dir"], np.float32).T)
    w_in = np.asarray(inputs["w_in"], np.float32)
    perm = np.empty(ENC, np.int64)
    for r in range(ENC):
        base = 0 if r < 30 else 10
        rr = r % 30
        perm[r] = (rr // 10) * 20 + base + (rr % 10)
    win_p = np.ascontiguousarray(w_in[perm])
    w8 = win_p.astype(E4)
    win8 = np.zeros((124, WIDTH), E4)
    win8[0:60] = w8
    win8[64:124] = w8

    w_hid = np.asarray(inputs["w_hid"], np.float32)
    HB = 7 * 2 * 128
    whid_hi = np.empty((128, 2, HB), E4)
    whid_cat = np.empty((128, 7 * 2 * WIDTH), np.float32)
    for l in range(7):
        W = w_hid[l]  # [256, 256]
        Wh = W.astype(E4)
        for m in range(2):
            blk = (l * 2 + m) * 128
            cols = slice(m * 128, (m + 1) * 128)
            whid_hi[:, 0, blk:blk + 128] = Wh[0:128, cols]
            whid_hi[:, 1, blk:blk + 128] = Wh[128:256, cols]
        for kc in range(2):
            whid_cat[:, (l * 2 + kc) * WIDTH:(l * 2 + kc + 1) * WIDTH] = \
                W[kc * 128:(kc + 1) * 128, :]

    whd = np.concatenate([np.asarray(inputs["w_rgb"], np.float32),
                          np.asarray(inputs["w_den"], np.float32)], axis=1)
    whd_h = whd.astype(E4)
    whd_hi = np.zeros((128, 2, 16), E4)
    whd_hi[:, 0, 0:4] = whd_h[0:128]
    whd_hi[:, 1, 0:4] = whd_h[128:256]
    whd_cat = np.empty((128, 8), np.float32)
    whd_cat[:, 0:4] = whd[0:128]
    whd_cat[:, 4:8] = whd[128:256]

    b_rgb = np.asarray(inputs["b_rgb"], np.float32)
    b_den = np.asarray(inputs["b_den"], np.float32)
    bca = np.zeros((128, 1), np.float32)
    bca[0:S] = 0.5 * b_rgb[0]
    bca[S:128] = 0.5 * b_rgb[1]
    bcb = np.full((S, 1), 0.5 * b_rgb[2], np.float32)
    bcd = np.full((S, 1), b_den[0], np.float32)

    # brep for 2-sample-packed fp32 encoder matmul: rows 32j+i of jpos map to
    # output rows 60*(j&1) + perm-row r with weight 2^k
    brepa01 = np.zeros((128, 124), np.float32)
    brepa23 = np.zeros((128, 124), np.float32)
    for r in range(ENC):
        rr = r % 30
        i, k = rr // 10, rr % 10
        brepa01[0 + i, r] = float(2.0 ** k)
        brepa01[32 + i, 64 + r] = float(2.0 ** k)
        brepa23[64 + i, r] = float(2.0 ** k)
        brepa23[96 + i, 64 + r] = float(2.0 ** k)
    mbias = np.zeros((124, 1), np.float32)
    mbias[30:60] = 0.25
    mbias[94:124] = 0.25
    b2v = np.zeros((124, 1), np.float32)
    b2v[30:60] = np.float32(np.pi / 2)
    b2v[94:124] = np.float32(np.pi / 2)
    # fast-path brep: rows scaled by 1/2pi, plus mbias via the ones-row 32j+3
    inv2pi = np.float64(1.0) / (2 * np.pi)
    brep01 = brepa01 * np.float32(inv2pi)
    brep23 = brepa23 * np.float32(inv2pi)
    for r in range(124):
        if mbias[r, 0]:
            if r < 64:
                brep01[3, r] = mbias[r, 0]
                brep23[67, r] = mbias[r, 0]
            else:
                brep01[35, r] = mbias[r, 0]
                brep23[99, r] = mbias[r, 0]
    # sin bias column: b2 - 2pi*mbias
    colc = (b2v - np.float32(2 * np.pi) * mbias).astype(np.float32)
    iota = np.arange(S, dtype=np.float32).reshape(S, 1)
    tris = (np.arange(S)[:, None] < np.arange(S)[None, :]).astype(np.float32)
    onesb = np.zeros((128, 2), np.float32)
    onesb[:64, 0] = 1.0
    onesb[64:, 1] = 1.0
    big = np.full((1, RPC), 1e10, np.float32)
    common = dict(win8=win8, win_32=win_p,
                  whid_hi=whid_hi, whid_32=whid_cat,
                  whd_hi=whd_hi, whd_32=whd_cat,
                  brep01=brep01, brep23=brep23,
                  brepa01=brepa01, brepa23=brepa23, colc=colc,
                  bca=bca, bcb=bcb, bcd=bcd, mbias=mbias, b2=b2v,
                  iota=iota, tris=tris, onesb=onesb, big=big)
    in_maps = []
    for c in range(N_CORES):
        sl = slice(c * RPC, (c + 1) * RPC)
        m = dict(common)
        m["jitter_t"] = np.ascontiguousarray(jt[:, sl])
        m["ray_pos_t"] = np.ascontiguousarray(rpt[:, sl])
        m["ray_dir_t"] = np.ascontiguousarray(rdt[:, sl])
        in_maps.append(m)
    return in_maps


def kernel(**inputs):
    global LAST_EXEC_NS
    from concourse.bass_utils import run_bass_kernel_spmd
    if "nc" not in _CACHE:
        _CACHE["nc"] = _build_nc()
    nc = _CACHE["nc"]
    in_maps = _prep(inputs)
    trace = bool(os.environ.get("KERNEL_TRACE"))
    res = run_bass_kernel_spmd(nc, in_maps, core_ids=list(range(N_CORES)),
                               trace=trace)
    LAST_EXEC_NS = getattr(res, "exec_time_ns", None)
    _CACHE["last_results"] = res.results
    _CACHE["last_res"] = res
    out = np.empty((N_CORES * RPC, 4), np.float32)
    for c in range(N_CORES):
        out[c * RPC:(c + 1) * RPC] = res.results[c]["out"].T
    return out


# revision 15
# speedup vs baseline: 1.0587x; 1.0073x over previous
"""NeRF render kernel for 8 TRN2 NeuronCores (pure data parallel over rays).

Per core: 512 rays x 64 samples, MLP width 256 x 8 layers + rgb/density heads,
then alpha-composite. Layout: activations [features(partition), rays(free)],
one sample-tile = 1 sample x 512 rays.

Fast path (samples 0-62): hidden layers run as fp8-e4m3 DoubleRow matmuls
(K=256 contracted per instruction); weights e4m3, activations quantized to
e4m3 directly by one merged [128,1024] relu per layer (all biases in this
problem are structurally zero). L0/heads fp8. The Fourier encoder is an fp32
matmul (2 samples packed per instruction) emitting pos*2^k/2pi + cos-phase,
magic-number rounding extracts n, and 2pi*(frac) + phase-bias feeds the HW
Sin. Sample 63 (tau = density*1e10 makes the density relu sign knife-edge)
runs in f32r end-to-end with a 2-step 2pi range reduction.

Schedule: tiles advance in 3-tile lockstep units (2 DR matmuls per tile per
layer on PE while the other tiles' relus run on Act/DVE, alternating), with
group prep (jpos on GpSimd, reduction, sin) and next-unit L0s emitted across
unit boundaries. PSUM: 4 rotating [128,1024] bank-pairs; heads write into the
tile's layer-7 pair; head outputs scatter via an SBUF stage + 2 DMAs/tile.
"""
import os
import numpy as np
import ml_dtypes

NB = 10
ENC = 60
WIDTH = 256
S = 64
RPC = 512  # rays per core
N_CORES = 8
NEAR, FAR = 0.1, 4.0
MAGIC = float(1.5 * 2**23)
INV2PI = float(1.0 / (2 * np.pi))
TWO_PI_F32 = float(np.float32(2 * np.pi))
P2HI = 6.28125  # 2pi hi word, exact in 8 mantissa bits
P2LO = float(2 * np.pi - 6.28125)

E4 = ml_dtypes.float8_e4m3
E5 = ml_dtypes.float8_e5m2

LAST_EXEC_NS = None
_CACHE = {}


def _build_nc():
    import concourse.bacc as bacc
    import concourse.tile as tile
    from concourse import mybir

    dt = mybir.dt
    AF = mybir.ActivationFunctionType
    ALU = mybir.AluOpType
    PM = mybir.MatmulPerfMode
    f32 = dt.float32
    f8h = dt.float8e4
    f8l = dt.float8e5
    f32r = dt.float32r

    nc = bacc.Bacc("TRN2", target_bir_lowering=False, debug=False,
                   num_devices=N_CORES)

    def din(name, shape, dtype=f32):
        return nc.dram_tensor(name, shape, dtype, kind="ExternalInput")

    HB = 7 * 2 * 128  # hidden DR weight block columns
    d_jit = din("jitter_t", [S, RPC])
    d_rp = din("ray_pos_t", [3, RPC])
    d_rd = din("ray_dir_t", [3, RPC])
    d_win8 = din("win8", [124, WIDTH], f8h)
    d_win_32 = din("win_32", [ENC, WIDTH], f32r)
    d_whid_hi = din("whid_hi", [128, 2, HB], f8h)
    d_whid_32 = din("whid_32", [128, 7 * 2 * WIDTH], f32r)
    d_whd_hi = din("whd_hi", [128, 2, 16], f8h)
    d_whd_32 = din("whd_32", [128, 8], f32r)
    d_brep01 = din("brep01", [128, 124])
    d_brep23 = din("brep23", [128, 124])
    d_brepa01 = din("brepa01", [128, 124])
    d_brepa23 = din("brepa23", [128, 124])
    d_colc = din("colc", [124, 1])
    d_bca = din("bca", [128, 1])
    d_bcb = din("bcb", [S, 1])
    d_bcd = din("bcd", [S, 1])
    d_mbias = din("mbias", [124, 1])
    d_b2 = din("b2", [124, 1])
    d_iota = din("iota", [S, 1])
    d_tris = din("tris", [S, S])
    d_onesb = din("onesb", [128, 2])
    d_big = din("big", [1, RPC])
    d_out = nc.dram_tensor("out", [4, RPC], f32, kind="ExternalOutput")

    # merged relu (biases are structurally zero in this problem): one
    # [128,1024] op per layer, alternating Act/DVE by (layer+tile) parity
    def engines(nc):
        def act(out, in_):
            nc.scalar.activation(out, in_, AF.Relu)

        def dve(out, in_):
            nc.vector.tensor_scalar(out, in_, 0.0, None, ALU.max)

        return (act, dve)

    with tile.TileContext(nc) as tc:
        with (
            tc.tile_pool(name="static", bufs=1) as sp,
            tc.tile_pool(name="act", bufs=7) as ap,
            tc.tile_pool(name="misc", bufs=3) as mp,
            tc.tile_pool(name="red", bufs=2) as rp,
            tc.tile_pool(name="comp", bufs=1) as cp,
            tc.tile_pool(name="ps_l", bufs=4, space="PSUM") as pl,
        ):
            def load(dram, shape, dtype, tag, eng=None):
                t = sp.tile(shape, dtype, tag=tag)
                (eng or nc.sync).dma_start(t[:], dram[:])
                return t

            jt = load(d_jit, [S, RPC], f32, "jt")
            iota = load(d_iota, [S, 1], f32, "iota")
            brep01 = load(d_brep01, [128, 124], f32, "brep01")
            brep23 = load(d_brep23, [128, 124], f32, "brep23")
            colc = load(d_colc, [124, 1], f32, "colc")
            win8 = load(d_win8, [124, WIDTH], f8h, "win8")
            whid_hi = load(d_whid_hi, [128, 2, HB], f8h, "whid_hi")
            whd_hi = load(d_whd_hi, [128, 2, 16], f8h, "whd_hi")
            whid_32 = load(d_whid_32, [128, 7 * 2 * WIDTH], f32r, "whid_32",
                           nc.gpsimd)
            brepa01 = load(d_brepa01, [128, 124], f32, "brepa01", nc.gpsimd)
            brepa23 = load(d_brepa23, [128, 124], f32, "brepa23", nc.gpsimd)
            win_32 = load(d_win_32, [ENC, WIDTH], f32r, "win_32", nc.gpsimd)
            whd_32 = load(d_whd_32, [128, 8], f32r, "whd_32", nc.gpsimd)
            bca = load(d_bca, [128, 1], f32, "bca", nc.gpsimd)
            bcb = load(d_bcb, [S, 1], f32, "bcb", nc.gpsimd)
            bcd = load(d_bcd, [S, 1], f32, "bcd", nc.gpsimd)
            mbias = load(d_mbias, [124, 1], f32, "mbias", nc.gpsimd)
            b2 = load(d_b2, [124, 1], f32, "b2", nc.gpsimd)
            tris = load(d_tris, [S, S], f32, "tris", nc.gpsimd)
            onesb = load(d_onesb, [128, 2], f32, "onesb", nc.gpsimd)

            rp128 = sp.tile([128, RPC], f32, tag="rp128")
            rd128 = sp.tile([128, RPC], f32, tag="rd128")
            nc.vector.memset(rp128[:], 1.0)
            nc.vector.memset(rd128[:], 0.0)
            for j in range(4):
                nc.sync.dma_start(rp128[32 * j:32 * j + 3, :], d_rp[:, :])
                nc.sync.dma_start(rd128[32 * j:32 * j + 3, :], d_rd[:, :])

            # depths = 0.1 + (3.9 * (idx + jitter)) / 64, exact fp32 op order
            ddtmp = sp.tile([S, RPC], f32, tag="ddtmp")
            nc.vector.tensor_scalar(ddtmp[:], jt[:], iota[:], 3.9, ALU.add, ALU.mult)
            dd = sp.tile([S, RPC], f32, tag="dd")
            nc.vector.tensor_scalar(dd[:], ddtmp[:], float(1.0 / 64), 0.1, ALU.mult, ALU.add)

            ddsh = cp.tile([S, RPC], f32, tag="ddsh")
            nc.sync.dma_start(ddsh[0:63, :], dd[1:64, :])
            nc.sync.dma_start(ddsh[63:64, :], d_big[:])
            delt = cp.tile([S, RPC], f32, tag="delt")
            nc.vector.tensor_sub(delt[:], ddsh[:], dd[:])

            # composite accumulation buffers
            cmpA = cp.tile([128, RPC], f32, tag="cmpA")  # rgb0 (0-63), rgb1 (64-127)
            cmpB = cp.tile([128, RPC], f32, tag="cmpB")  # rgb2 (0-63), den (64-127)

            ENG = engines(nc)

            def prep_group(g):
                """jpos, enc matmuls, range reduction, sin for samples 4g..4g+3.

                Returns (enc8, enc32) -- enc8 [120,1024] e4m3; enc32 [60,512]
                f32 only for the last group (sample 63)."""
                s0 = 4 * g
                acc = (g == 15)
                dd4 = mp.tile([128, RPC], f32, tag="dd4")
                for i in range(3):
                    nc.sync.dma_start(dd4[i::32, :], dd[s0:s0 + 4, :])
                jeng = nc.vector if g < 2 else nc.gpsimd
                jtmp = mp.tile([128, RPC], f32, tag="jtmp")
                jeng.tensor_mul(jtmp[:], dd4[:], rd128[:])
                jpos = mp.tile([128, RPC], f32, tag="jpos")
                jeng.tensor_add(jpos[:], jtmp[:], rp128[:])

                pe = pl.tile([128, 1024], f32, tag="lp")
                b01, b23 = (brepa01, brepa23) if acc else (brep01, brep23)
                nc.tensor.matmul(pe[0:124, 0:512], b01[:], jpos[:],
                                 start=True, stop=True)
                nc.tensor.matmul(pe[0:124, 512:1024], b23[:], jpos[:],
                                 start=True, stop=True)
                xb = pe[0:124, :]
                enc8 = ap.tile([124, 1024], f8h, tag="enc8")
                if not acc:
                    # xb = pos*2^k/2pi + mbias; n = round(xb); r = 2pi*(xb - n)
                    # - 2pi*mbias + b2 folded into sin bias column
                    rn = rp.tile([124, 1024], f32, tag="red_n")
                    nc.vector.tensor_scalar(rn[:], xb, MAGIC, MAGIC,
                                            ALU.add, ALU.subtract)
                    ru = rp.tile([124, 1024], f32, tag="red_u")
                    nc.scalar.activation(ru[:], rn[:], AF.Copy, scale=-1.0)
                    rr = rp.tile([124, 1024], f32, tag="red_r")
                    nc.vector.tensor_tensor(rr[:], xb, ru[:], ALU.add)
                    nc.scalar.activation(enc8[:], rr[:], AF.Sin, bias=colc[:],
                                         scale=TWO_PI_F32)
                    return enc8, None
                # acc group (sample 63): original high-precision path
                rt = rp.tile([124, 1024], f32, tag="red_t")
                nc.vector.tensor_scalar(rt[:], xb, INV2PI, mbias[:], ALU.mult, ALU.add)
                rn = rp.tile([124, 1024], f32, tag="red_n")
                nc.vector.tensor_scalar(rn[:], rt[:], MAGIC, MAGIC, ALU.add, ALU.subtract)
                ru = rp.tile([124, 1024], f32, tag="red_u")
                nc.scalar.activation(ru[:], rn[:], AF.Copy, scale=-P2HI)
                rr = rp.tile([124, 1024], f32, tag="red_r")
                nc.vector.scalar_tensor_tensor(rr[:], ru[:], b2[:], xb, ALU.add, ALU.add)
                ru2 = rp.tile([124, 1024], f32, tag="red_u2")
                nc.vector.tensor_scalar(ru2[:], rn[:], P2LO, None, ALU.mult)
                nc.vector.tensor_sub(rr[:], rr[:], ru2[:])
                nc.scalar.activation(enc8[:], rr[:], AF.Sin)
                enc32 = mp.tile([ENC, RPC], f32r, tag="enc32")
                nc.scalar.activation(enc32[:], rr[64:124, 512:1024], AF.Sin)
                return enc8, enc32

            def enc_slice(enc8, s):
                r0 = 64 * (s & 1)
                c0 = 512 * ((s >> 1) & 1)
                return enc8[r0:r0 + 60, c0:c0 + 512]

            def mlp8_layer(l, x_in, x_out, todd):
                """One fp8 hidden layer: 2 DR matmuls + 1 merged relu."""
                p = pl.tile([128, 1024], f32, tag="lp")
                for m in range(2):
                    blk = ((l - 1) * 2 + m) * 128
                    nc.tensor.matmul(p[:, 512 * m:512 * m + 512],
                                     whid_hi[:, :, blk:blk + 128], x_in[:, :, :],
                                     start=True, stop=True, perf_mode=PM.DoubleRow)
                ENG[(l + todd) & 1](x_out[:, :, :], p[:, 0:1024])
                return p

            def mlp32_layer(l, x_in, x_out, todd):
                """One f32r hidden layer (sample 63 path)."""
                p = pl.tile([128, 1024], f32, tag="lp")
                for m in range(2):
                    for kc in range(2):
                        base = ((l - 1) * 2 + kc) * WIDTH + m * 128
                        nc.tensor.matmul(p[:, 512 * m:512 * m + 512],
                                         whid_32[:, base:base + 128],
                                         x_in[:, 512 * kc:512 * kc + 512],
                                         start=(kc == 0), stop=(kc == 1))
                ENG[(l + todd) & 1](x_out[:], p[:, 0:1024])
                return p

            def l0_stage8(enc8, s, i):
                es = enc_slice(enc8, s)
                r0 = 64 * (s & 1)
                p0 = pl.tile([128, 1024], f32, tag="lp")
                for m in range(2):
                    nc.tensor.matmul(p0[:, 512 * m:512 * m + 512],
                                     win8[r0:r0 + 60, 128 * m:128 * m + 128], es,
                                     start=True, stop=True)
                x = ap.tile([128, 2, 512], f8h, tag="x8")
                ENG[i & 1](x[:, :, :], p0[:, 0:1024])
                return x

            def layer_stage8(l, x, i):
                xn = ap.tile([128, 2, 512], f8h, tag="x8")
                p = mlp8_layer(l, x, xn, i)
                return xn if l < 7 else (xn, p)

            def head_stage8(x, p7, s):
                nc.tensor.matmul(p7[0:16, 0:512], whd_hi[:, :, :],
                                 x[:, :, :], start=True, stop=True,
                                 perf_mode=PM.DoubleRow)

            def l0_stage32(enc32, i):
                p0 = pl.tile([128, 1024], f32, tag="lp")
                nc.tensor.matmul(p0[:, 0:512], win_32[:, 0:128], enc32[:],
                                 start=True, stop=True)
                nc.tensor.matmul(p0[:, 512:1024], win_32[:, 128:256], enc32[:],
                                 start=True, stop=True)
                x = mp.tile([128, 1024], f32r, tag="x32")
                ENG[i & 1](x[:], p0[:, 0:1024])
                return x

            def layer_stage32(l, x, i):
                xn = mp.tile([128, 1024], f32r, tag="x32")
                p = mlp32_layer(l, x, xn, i)
                return xn if l < 7 else (xn, p)

            def head_stage32(x, p7, s):
                for kc in range(2):
                    nc.tensor.matmul(p7[0:4, 0:512],
                                     whd_32[:, kc * 4:kc * 4 + 4],
                                     x[:, 512 * kc:512 * kc + 512],
                                     start=(kc == 0), stop=(kc == 1))

            def scatter_tile(s, p7, todd):
                stg = mp.tile([16, RPC], f32, tag="stg")
                nc.scalar.copy(stg[:], p7[0:16, 0:512])
                nc.sync.dma_start(cmpA[s::64, :], stg[0:2, :])
                nc.sync.dma_start(cmpB[s::64, :], stg[2:4, :])

            def l0_unit(unit, preps):
                st = {}
                for i, s in enumerate(unit):
                    enc8, enc32 = preps[s >> 2]
                    if s == 63:
                        st[s] = (layer_stage32, head_stage32,
                                 l0_stage32(enc32, i))
                    else:
                        st[s] = (layer_stage8, head_stage8,
                                 l0_stage8(enc8, s, i))
                return st

            def body_unit(unit, st):
                for l in range(1, 8):
                    for i, s in enumerate(unit):
                        fl, fh, x = st[s]
                        st[s] = (fl, fh, fl(l, x, i))

            def finish_unit(unit, st):
                for s in unit:
                    fl, fh, (x, p7) = st[s]
                    fh(x, p7, s)
                    scatter_tile(s, p7, (s // 3) & 1)

            units = [tuple(range(t, t + 3)) for t in range(0, 60, 3)]
            units.append((60, 61, 62, 63))
            preps = {}
            next_prep = [0]

            def ensure_prep(upto):
                while next_prep[0] <= min(upto, 15):
                    g = next_prep[0]
                    preps[g] = prep_group(g)
                    next_prep[0] += 1

            ensure_prep(1)
            st = l0_unit(units[0], preps)
            for u, unit in enumerate(units):
                body_unit(unit, st)
                ensure_prep((max(unit) + 12) >> 2)
                if u + 1 < len(units):
                    st_next = l0_unit(units[u + 1], preps)
                else:
                    st_next = None
                finish_unit(unit, st)
                st = st_next

            # ---- head activations ----
            tmpa = cp.tile([128, RPC], f32, tag="tmpa")
            nc.scalar.activation(tmpa[:], cmpA[:], AF.Tanh, bias=bca[:], scale=0.5)
            nc.vector.tensor_scalar(cmpA[:], tmpa[:], 0.5, 0.5, ALU.mult, ALU.add)
            tmpb = cp.tile([S, RPC], f32, tag="tmpb")
            nc.scalar.activation(tmpb[:], cmpB[0:S, :], AF.Tanh, bias=bcb[:], scale=0.5)
            denr = cp.tile([S, RPC], f32, tag="denr")
            nc.vector.tensor_scalar(denr[:], cmpB[S:128, :], bcd[:], 0.0,
                                    ALU.add, ALU.max)
            nc.vector.tensor_scalar(cmpB[0:S, :], tmpb[:], 0.5, 0.5, ALU.mult, ALU.add)

            # ---- volume rendering composite ----
            tau = cp.tile([S, RPC], f32, tag="tau")
            nc.vector.tensor_mul(tau[:], denr[:], delt[:])
            pep = pl.tile([128, 1024], f32, tag="lp")
            nc.tensor.matmul(pep[0:S, 0:512], tris[:], tau[:], start=True, stop=True)
            inc = cp.tile([S, RPC], f32, tag="inc")
            nc.vector.tensor_add(inc[:], pep[0:S, 0:512], tau[:])
            exc2 = cp.tile([S, RPC], f32, tag="exc2")
            nc.vector.tensor_sub(exc2[:], inc[:], tau[:])
            trans = cp.tile([S, RPC], f32, tag="trans")
            nc.scalar.activation(trans[:], exc2[:], AF.Exp, scale=-1.0)
            ee = cp.tile([S, RPC], f32, tag="ee")
            nc.scalar.activation(ee[:], tau[:], AF.Exp, scale=-1.0)
            alpha = cp.tile([S, RPC], f32, tag="alpha")
            nc.vector.tensor_scalar(alpha[:], ee[:], -1.0, 1.0, ALU.mult, ALU.add)
            wt = cp.tile([S, RPC], f32, tag="wt")
            nc.vector.tensor_mul(wt[:], alpha[:], trans[:])
            w2 = cp.tile([128, RPC], f32, tag="w2")
            nc.sync.dma_start(w2[0:S, :], wt[:])
            nc.sync.dma_start(w2[S:128, :], wt[:])
            nc.sync.dma_start(cmpB[S:128, :], dd[:])
            wa = cp.tile([128, RPC], f32, tag="wa")
            nc.vector.tensor_mul(wa[:], w2[:], cmpA[:])
            wb = cp.tile([128, RPC], f32, tag="wb")
            nc.vector.tensor_mul(wb[:], w2[:], cmpB[:])
            nc.tensor.matmul(pep[0:2, 512:1024], onesb[:], wa[:], start=True,
                             stop=True, tile_position=(0, 0))
            nc.tensor.matmul(pep[32:34, 512:1024], onesb[:], wb[:], start=True,
                             stop=True, tile_position=(0, 32))
            outsb = cp.tile([S, RPC], f32, tag="outsb")
            nc.vector.tensor_copy(outsb[0:2, :], pep[0:2, 512:1024])
            nc.vector.tensor_copy(outsb[32:34, :], pep[32:34, 512:1024])
            nc.sync.dma_start(d_out[0:2, :], outsb[0:2, :])
            nc.sync.dma_start(d_out[2:4, :], outsb[32:34, :])

    nc.compile()
    return nc


def _prep(inputs):
    jt = np.ascontiguousarray(np.asarray(inputs["jitter"], np.float32).T)
    rpt = np.ascontiguousarray(np.asarray(inputs["ray_pos"], np.float32).T)
    rdt = np.ascontiguousarray(np.asarray(inputs["ray_dir"], np.float32).T)
    w_in = np.asarray(inputs["w_in"], np.float32)
    perm = np.empty(ENC, np.int64)
    for r in range(ENC):
        base = 0 if r < 30 else 10
        rr = r % 30
        perm[r] = (rr // 10) * 20 + base + (rr % 10)
    win_p = np.ascontiguousarray(w_in[perm])
    w8 = win_p.astype(E4)
    win8 = np.zeros((124, WIDTH), E4)
    win8[0:60] = w8
    win8[64:124] = w8

    w_hid = np.asarray(inputs["w_hid"], np.float32)
    HB = 7 * 2 * 128
    whid_hi = np.empty((128, 2, HB), E4)
    whid_cat = np.empty((128, 7 * 2 * WIDTH), np.float32)
    for l in range(7):
        W = w_hid[l]  # [256, 256]
        Wh = W.astype(E4)
        for m in range(2):
            blk = (l * 2 + m) * 128
            cols = slice(m * 128, (m + 1) * 128)
            whid_hi[:, 0, blk:blk + 128] = Wh[0:128, cols]
            whid_hi[:, 1, blk:blk + 128] = Wh[128:256, cols]
        for kc in range(2):
            whid_cat[:, (l * 2 + kc) * WIDTH:(l * 2 + kc + 1) * WIDTH] = \
                W[kc * 128:(kc + 1) * 128, :]

    whd = np.concatenate([np.asarray(inputs["w_rgb"], np.float32),
                          np.asarray(inputs["w_den"], np.float32)], axis=1)
    whd_h = whd.astype(E4)
    whd_hi = np.zeros((128, 2, 16), E4)
    whd_hi[:, 0, 0:4] = whd_h[0:128]
    whd_hi[:, 1, 0:4] = whd_h[128:256]
    whd_cat = np.empty((128, 8), np.float32)
    whd_cat[:, 0:4] = whd[0:128]
    whd_cat[:, 4:8] = whd[128:256]

    b_rgb = np.asarray(inputs["b_rgb"], np.float32)
    b_den = np.asarray(inputs["b_den"], np.float32)
    bca = np.zeros((128, 1), np.float32)
    bca[0:S] = 0.5 * b_rgb[0]
    bca[S:128] = 0.5 * b_rgb[1]
    bcb = np.full((S, 1), 0.5 * b_rgb[2], np.float32)
    bcd = np.full((S, 1), b_den[0], np.float32)

    # brep for 2-sample-packed fp32 encoder matmul: rows 32j+i of jpos map to
    # output rows 60*(j&1) + perm-row r with weight 2^k
    brepa01 = np.zeros((128, 124), np.float32)
    brepa23 = np.zeros((128, 124), np.float32)
    for r in range(ENC):
        rr = r % 30
        i, k = rr // 10, rr % 10
        brepa01[0 + i, r] = float(2.0 ** k)
        brepa01[32 + i, 64 + r] = float(2.0 ** k)
        brepa23[64 + i, r] = float(2.0 ** k)
        brepa23[96 + i, 64 + r] = float(2.0 ** k)
    mbias = np.zeros((124, 1), np.float32)
    mbias[30:60] = 0.25
    mbias[94:124] = 0.25
    b2v = np.zeros((124, 1), np.float32)
    b2v[30:60] = np.float32(np.pi / 2)
    b2v[94:124] = np.float32(np.pi / 2)
    # fast-path brep: rows scaled by 1/2pi, plus mbias via the ones-row 32j+3
    inv2pi = np.float64(1.0) / (2 * np.pi)
    brep01 = brepa01 * np.float32(inv2pi)
    brep23 = brepa23 * np.float32(inv2pi)
    for r in range(124):
        if mbias[r, 0]:
            if r < 64:
                brep01[3, r] = mbias[r, 0]
                brep23[67, r] = mbias[r, 0]
            else:
                brep01[35, r] = mbias[r, 0]
                brep23[99, r] = mbias[r, 0]
    # sin bias column: b2 - 2pi*mbias
    colc = (b2v - np.float32(2 * np.pi) * mbias).astype(np.float32)
    iota = np.arange(S, dtype=np.float32).reshape(S, 1)
    tris = (np.arange(S)[:, None] < np.arange(S)[None, :]).astype(np.float32)
    onesb = np.zeros((128, 2), np.float32)
    onesb[:64, 0] = 1.0
    onesb[64:, 1] = 1.0
    big = np.full((1, RPC), 1e10, np.float32)
    common = dict(win8=win8, win_32=win_p,
                  whid_hi=whid_hi, whid_32=whid_cat,
                  whd_hi=whd_hi, whd_32=whd_cat,
                  brep01=brep01, brep23=brep23,
                  brepa01=brepa01, brepa23=brepa23, colc=colc,
                  bca=bca, bcb=bcb, bcd=bcd, mbias=mbias, b2=b2v,
                  iota=iota, tris=tris, onesb=onesb, big=big)
    in_maps = []
    for c in range(N_CORES):
        sl = slice(c * RPC, (c + 1) * RPC)
        m = dict(common)
        m["jitter_t"] = np.ascontiguousarray(jt[:, sl])
        m["ray_pos_t"] = np.ascontiguousarray(rpt[:, sl])
        m["ray_dir_t"] = np.ascontiguousarray(rdt[:, sl])
        in_maps.append(m)
    return in_maps


def kernel(**inputs):
    global LAST_EXEC_NS
    from concourse.bass_utils import run_bass_kernel_spmd
    if "nc" not in _CACHE:
        _CACHE["nc"] = _build_nc()
    nc = _CACHE["nc"]
    in_maps = _prep(inputs)
    trace = bool(os.environ.get("KERNEL_TRACE"))
    res = run_bass_kernel_spmd(nc, in_maps, core_ids=list(range(N_CORES)),
                               trace=trace)
    LAST_EXEC_NS = getattr(res, "exec_time_ns", None)
    _CACHE["last_results"] = res.results
    _CACHE["last_res"] = res
    out = np.empty((N_CORES * RPC, 4), np.float32)
    for c in range(N_CORES):
        out[c * RPC:(c + 1) * RPC] = res.results[c]["out"].T
    return out


# revision 16
# speedup vs baseline: 1.0604x; 1.0016x over previous
"""NeRF render kernel for 8 TRN2 NeuronCores (pure data parallel over rays).

Per core: 512 rays x 64 samples, MLP width 256 x 8 layers + rgb/density heads,
then alpha-composite. Layout: activations [features(partition), rays(free)],
one sample-tile = 1 sample x 512 rays.

Fast path (samples 0-62): hidden layers run as fp8-e4m3 DoubleRow matmuls
(K=256 contracted per instruction); weights e4m3, activations quantized to
e4m3 directly by one merged [128,1024] relu per layer (all biases in this
problem are structurally zero). L0/heads fp8. The Fourier encoder is an fp32
matmul (2 samples packed per instruction) emitting pos*2^k/2pi + cos-phase,
magic-number rounding extracts n, and 2pi*(frac) + phase-bias feeds the HW
Sin. Sample 63 (tau = density*1e10 makes the density relu sign knife-edge)
runs in f32r end-to-end with a 2-step 2pi range reduction.

Schedule: tiles advance in 3-tile lockstep units (2 DR matmuls per tile per
layer on PE while the other tiles' relus run on Act/DVE, alternating), with
group prep (jpos on GpSimd, reduction, sin) and next-unit L0s emitted across
unit boundaries. PSUM: 4 rotating [128,1024] bank-pairs; heads write into the
tile's layer-7 pair; head outputs scatter via an SBUF stage + 2 DMAs/tile.
"""
import os
import numpy as np
import ml_dtypes

NB = 10
ENC = 60
WIDTH = 256
S = 64
RPC = 512  # rays per core
N_CORES = 8
NEAR, FAR = 0.1, 4.0
MAGIC = float(1.5 * 2**23)
INV2PI = float(1.0 / (2 * np.pi))
TWO_PI_F32 = float(np.float32(2 * np.pi))
P2HI = 6.28125  # 2pi hi word, exact in 8 mantissa bits
P2LO = float(2 * np.pi - 6.28125)

E4 = ml_dtypes.float8_e4m3
E5 = ml_dtypes.float8_e5m2

LAST_EXEC_NS = None
_CACHE = {}


def _build_nc():
    import concourse.bacc as bacc
    import concourse.tile as tile
    from concourse import mybir

    dt = mybir.dt
    AF = mybir.ActivationFunctionType
    ALU = mybir.AluOpType
    PM = mybir.MatmulPerfMode
    f32 = dt.float32
    f8h = dt.float8e4
    f8l = dt.float8e5
    f32r = dt.float32r

    nc = bacc.Bacc("TRN2", target_bir_lowering=False, debug=False,
                   num_devices=N_CORES)

    def din(name, shape, dtype=f32):
        return nc.dram_tensor(name, shape, dtype, kind="ExternalInput")

    HB = 7 * 2 * 128  # hidden DR weight block columns
    d_jit = din("jitter_t", [S, RPC])
    d_rp = din("ray_pos_t", [3, RPC])
    d_rd = din("ray_dir_t", [3, RPC])
    d_win8 = din("win8", [124, WIDTH], f8h)
    d_win_32 = din("win_32", [ENC, WIDTH], f32r)
    d_whid_hi = din("whid_hi", [128, 2, HB], f8h)
    d_whid_32 = din("whid_32", [128, 7 * 2 * WIDTH], f32r)
    d_whd_hi = din("whd_hi", [128, 2, 16], f8h)
    d_whd_32 = din("whd_32", [128, 8], f32r)
    d_brep01 = din("brep01", [128, 124])
    d_brep23 = din("brep23", [128, 124])
    d_brepa01 = din("brepa01", [128, 124])
    d_brepa23 = din("brepa23", [128, 124])
    d_colc = din("colc", [124, 1])
    d_bca = din("bca", [128, 1])
    d_bcb = din("bcb", [S, 1])
    d_bcd = din("bcd", [S, 1])
    d_mbias = din("mbias", [124, 1])
    d_b2 = din("b2", [124, 1])
    d_iota = din("iota", [S, 1])
    d_tris = din("tris", [S, S])
    d_onesb = din("onesb", [128, 2])
    d_big = din("big", [1, RPC])
    d_out = nc.dram_tensor("out", [4, RPC], f32, kind="ExternalOutput")

    # merged relu (biases are structurally zero in this problem): one
    # [128,1024] op per layer, alternating Act/DVE by (layer+tile) parity
    def engines(nc):
        def act(out, in_):
            nc.scalar.activation(out, in_, AF.Relu)

        def dve(out, in_):
            nc.vector.tensor_scalar(out, in_, 0.0, None, ALU.max)

        return (act, dve)

    with tile.TileContext(nc) as tc:
        with (
            tc.tile_pool(name="static", bufs=1) as sp,
            tc.tile_pool(name="act", bufs=7) as ap,
            tc.tile_pool(name="misc", bufs=4) as mp,
            tc.tile_pool(name="red", bufs=3) as rp,
            tc.tile_pool(name="comp", bufs=1) as cp,
            tc.tile_pool(name="ps_l", bufs=4, space="PSUM") as pl,
        ):
            def load(dram, shape, dtype, tag, eng=None):
                t = sp.tile(shape, dtype, tag=tag)
                (eng or nc.sync).dma_start(t[:], dram[:])
                return t

            jt = load(d_jit, [S, RPC], f32, "jt")
            iota = load(d_iota, [S, 1], f32, "iota")
            brep01 = load(d_brep01, [128, 124], f32, "brep01")
            brep23 = load(d_brep23, [128, 124], f32, "brep23")
            colc = load(d_colc, [124, 1], f32, "colc")
            win8 = load(d_win8, [124, WIDTH], f8h, "win8")
            whid_hi = load(d_whid_hi, [128, 2, HB], f8h, "whid_hi")
            whd_hi = load(d_whd_hi, [128, 2, 16], f8h, "whd_hi")
            whid_32 = load(d_whid_32, [128, 7 * 2 * WIDTH], f32r, "whid_32",
                           nc.gpsimd)
            brepa01 = load(d_brepa01, [128, 124], f32, "brepa01", nc.gpsimd)
            brepa23 = load(d_brepa23, [128, 124], f32, "brepa23", nc.gpsimd)
            win_32 = load(d_win_32, [ENC, WIDTH], f32r, "win_32", nc.gpsimd)
            whd_32 = load(d_whd_32, [128, 8], f32r, "whd_32", nc.gpsimd)
            bca = load(d_bca, [128, 1], f32, "bca", nc.gpsimd)
            bcb = load(d_bcb, [S, 1], f32, "bcb", nc.gpsimd)
            bcd = load(d_bcd, [S, 1], f32, "bcd", nc.gpsimd)
            mbias = load(d_mbias, [124, 1], f32, "mbias", nc.gpsimd)
            b2 = load(d_b2, [124, 1], f32, "b2", nc.gpsimd)
            tris = load(d_tris, [S, S], f32, "tris", nc.gpsimd)
            onesb = load(d_onesb, [128, 2], f32, "onesb", nc.gpsimd)

            rp128 = sp.tile([128, RPC], f32, tag="rp128")
            rd128 = sp.tile([128, RPC], f32, tag="rd128")
            nc.vector.memset(rp128[:], 1.0)
            nc.vector.memset(rd128[:], 0.0)
            for j in range(4):
                nc.sync.dma_start(rp128[32 * j:32 * j + 3, :], d_rp[:, :])
                nc.sync.dma_start(rd128[32 * j:32 * j + 3, :], d_rd[:, :])

            # depths = 0.1 + (3.9 * (idx + jitter)) / 64, exact fp32 op order
            ddtmp = sp.tile([S, RPC], f32, tag="ddtmp")
            nc.vector.tensor_scalar(ddtmp[:], jt[:], iota[:], 3.9, ALU.add, ALU.mult)
            dd = sp.tile([S, RPC], f32, tag="dd")
            nc.vector.tensor_scalar(dd[:], ddtmp[:], float(1.0 / 64), 0.1, ALU.mult, ALU.add)

            ddsh = cp.tile([S, RPC], f32, tag="ddsh")
            nc.sync.dma_start(ddsh[0:63, :], dd[1:64, :])
            nc.sync.dma_start(ddsh[63:64, :], d_big[:])
            delt = cp.tile([S, RPC], f32, tag="delt")
            nc.vector.tensor_sub(delt[:], ddsh[:], dd[:])

            # composite accumulation buffers
            cmpA = cp.tile([128, RPC], f32, tag="cmpA")  # rgb0 (0-63), rgb1 (64-127)
            cmpB = cp.tile([128, RPC], f32, tag="cmpB")  # rgb2 (0-63), den (64-127)

            ENG = engines(nc)

            def prep_group(g):
                """jpos, enc matmuls, range reduction, sin for samples 4g..4g+3.

                Returns (enc8, enc32) -- enc8 [120,1024] e4m3; enc32 [60,512]
                f32 only for the last group (sample 63)."""
                s0 = 4 * g
                acc = (g == 15)
                dd4 = mp.tile([128, RPC], f32, tag="dd4")
                for i in range(3):
                    nc.sync.dma_start(dd4[i::32, :], dd[s0:s0 + 4, :])
                jeng = nc.vector if g < 2 else nc.gpsimd
                jtmp = mp.tile([128, RPC], f32, tag="jtmp")
                jeng.tensor_mul(jtmp[:], dd4[:], rd128[:])
                jpos = mp.tile([128, RPC], f32, tag="jpos")
                jeng.tensor_add(jpos[:], jtmp[:], rp128[:])

                pe = pl.tile([128, 1024], f32, tag="lp")
                b01, b23 = (brepa01, brepa23) if acc else (brep01, brep23)
                nc.tensor.matmul(pe[0:124, 0:512], b01[:], jpos[:],
                                 start=True, stop=True)
                nc.tensor.matmul(pe[0:124, 512:1024], b23[:], jpos[:],
                                 start=True, stop=True)
                xb = pe[0:124, :]
                enc8 = ap.tile([124, 1024], f8h, tag="enc8")
                if not acc:
                    # xb = pos*2^k/2pi + mbias; n = round(xb); r = 2pi*(xb - n)
                    # - 2pi*mbias + b2 folded into sin bias column
                    rn = rp.tile([124, 1024], f32, tag="red_n")
                    nc.vector.tensor_scalar(rn[:], xb, MAGIC, MAGIC,
                                            ALU.add, ALU.subtract)
                    ru = rp.tile([124, 1024], f32, tag="red_u")
                    nc.scalar.activation(ru[:], rn[:], AF.Copy, scale=-1.0)
                    rr = rp.tile([124, 1024], f32, tag="red_r")
                    nc.vector.tensor_tensor(rr[:], xb, ru[:], ALU.add)
                    nc.scalar.activation(enc8[:], rr[:], AF.Sin, bias=colc[:],
                                         scale=TWO_PI_F32)
                    return enc8, None
                # acc group (sample 63): original high-precision path
                rt = rp.tile([124, 1024], f32, tag="red_t")
                nc.vector.tensor_scalar(rt[:], xb, INV2PI, mbias[:], ALU.mult, ALU.add)
                rn = rp.tile([124, 1024], f32, tag="red_n")
                nc.vector.tensor_scalar(rn[:], rt[:], MAGIC, MAGIC, ALU.add, ALU.subtract)
                ru = rp.tile([124, 1024], f32, tag="red_u")
                nc.scalar.activation(ru[:], rn[:], AF.Copy, scale=-P2HI)
                rr = rp.tile([124, 1024], f32, tag="red_r")
                nc.vector.scalar_tensor_tensor(rr[:], ru[:], b2[:], xb, ALU.add, ALU.add)
                ru2 = rp.tile([124, 1024], f32, tag="red_u2")
                nc.vector.tensor_scalar(ru2[:], rn[:], P2LO, None, ALU.mult)
                nc.vector.tensor_sub(rr[:], rr[:], ru2[:])
                nc.scalar.activation(enc8[:], rr[:], AF.Sin)
                enc32 = mp.tile([ENC, RPC], f32r, tag="enc32")
                nc.scalar.activation(enc32[:], rr[64:124, 512:1024], AF.Sin)
                return enc8, enc32

            def enc_slice(enc8, s):
                r0 = 64 * (s & 1)
                c0 = 512 * ((s >> 1) & 1)
                return enc8[r0:r0 + 60, c0:c0 + 512]

            def mlp8_layer(l, x_in, x_out, todd):
                """One fp8 hidden layer: 2 DR matmuls + 1 merged relu."""
                p = pl.tile([128, 1024], f32, tag="lp")
                for m in range(2):
                    blk = ((l - 1) * 2 + m) * 128
                    nc.tensor.matmul(p[:, 512 * m:512 * m + 512],
                                     whid_hi[:, :, blk:blk + 128], x_in[:, :, :],
                                     start=True, stop=True, perf_mode=PM.DoubleRow)
                ENG[(l + todd) & 1](x_out[:, :, :], p[:, 0:1024])
                return p

            def mlp32_layer(l, x_in, x_out, todd):
                """One f32r hidden layer (sample 63 path)."""
                p = pl.tile([128, 1024], f32, tag="lp")
                for m in range(2):
                    for kc in range(2):
                        base = ((l - 1) * 2 + kc) * WIDTH + m * 128
                        nc.tensor.matmul(p[:, 512 * m:512 * m + 512],
                                         whid_32[:, base:base + 128],
                                         x_in[:, 512 * kc:512 * kc + 512],
                                         start=(kc == 0), stop=(kc == 1))
                ENG[(l + todd) & 1](x_out[:], p[:, 0:1024])
                return p

            def l0_stage8(enc8, s, i):
                es = enc_slice(enc8, s)
                r0 = 64 * (s & 1)
                p0 = pl.tile([128, 1024], f32, tag="lp")
                for m in range(2):
                    nc.tensor.matmul(p0[:, 512 * m:512 * m + 512],
                                     win8[r0:r0 + 60, 128 * m:128 * m + 128], es,
                                     start=True, stop=True)
                x = ap.tile([128, 2, 512], f8h, tag="x8")
                ENG[i & 1](x[:, :, :], p0[:, 0:1024])
                return x

            def layer_stage8(l, x, i):
                xn = ap.tile([128, 2, 512], f8h, tag="x8")
                p = mlp8_layer(l, x, xn, i)
                return xn if l < 7 else (xn, p)

            def head_stage8(x, p7, s):
                nc.tensor.matmul(p7[0:16, 0:512], whd_hi[:, :, :],
                                 x[:, :, :], start=True, stop=True,
                                 perf_mode=PM.DoubleRow)

            def l0_stage32(enc32, i):
                p0 = pl.tile([128, 1024], f32, tag="lp")
                nc.tensor.matmul(p0[:, 0:512], win_32[:, 0:128], enc32[:],
                                 start=True, stop=True)
                nc.tensor.matmul(p0[:, 512:1024], win_32[:, 128:256], enc32[:],
                                 start=True, stop=True)
                x = mp.tile([128, 1024], f32r, tag="x32")
                ENG[i & 1](x[:], p0[:, 0:1024])
                return x

            def layer_stage32(l, x, i):
                xn = mp.tile([128, 1024], f32r, tag="x32")
                p = mlp32_layer(l, x, xn, i)
                return xn if l < 7 else (xn, p)

            def head_stage32(x, p7, s):
                for kc in range(2):
                    nc.tensor.matmul(p7[0:4, 0:512],
                                     whd_32[:, kc * 4:kc * 4 + 4],
                                     x[:, 512 * kc:512 * kc + 512],
                                     start=(kc == 0), stop=(kc == 1))

            def scatter_tile(s, p7, todd):
                stg = mp.tile([16, RPC], f32, tag="stg")
                nc.scalar.copy(stg[:], p7[0:16, 0:512])
                nc.sync.dma_start(cmpA[s::64, :], stg[0:2, :])
                nc.sync.dma_start(cmpB[s::64, :], stg[2:4, :])

            def l0_unit(unit, preps):
                st = {}
                for i, s in enumerate(unit):
                    enc8, enc32 = preps[s >> 2]
                    if s == 63:
                        st[s] = (layer_stage32, head_stage32,
                                 l0_stage32(enc32, i))
                    else:
                        st[s] = (layer_stage8, head_stage8,
                                 l0_stage8(enc8, s, i))
                return st

            def body_unit(unit, st):
                for l in range(1, 8):
                    for i, s in enumerate(unit):
                        fl, fh, x = st[s]
                        st[s] = (fl, fh, fl(l, x, i))

            def finish_unit(unit, st):
                for s in unit:
                    fl, fh, (x, p7) = st[s]
                    fh(x, p7, s)
                    scatter_tile(s, p7, (s // 3) & 1)

            units = [tuple(range(t, t + 3)) for t in range(0, 60, 3)]
            units.append((60, 61, 62, 63))
            preps = {}
            next_prep = [0]

            def ensure_prep(upto):
                while next_prep[0] <= min(upto, 15):
                    g = next_prep[0]
                    preps[g] = prep_group(g)
                    next_prep[0] += 1

            ensure_prep(1)
            st = l0_unit(units[0], preps)
            for u, unit in enumerate(units):
                body_unit(unit, st)
                ensure_prep((max(unit) + 16) >> 2)
                if u + 1 < len(units):
                    st_next = l0_unit(units[u + 1], preps)
                else:
                    st_next = None
                finish_unit(unit, st)
                st = st_next

            # ---- head activations ----
            tmpa = cp.tile([128, RPC], f32, tag="tmpa")
            nc.scalar.activation(tmpa[:], cmpA[:], AF.Tanh, bias=bca[:], scale=0.5)
            nc.vector.tensor_scalar(cmpA[:], tmpa[:], 0.5, 0.5, ALU.mult, ALU.add)
            tmpb = cp.tile([S, RPC], f32, tag="tmpb")
            nc.scalar.activation(tmpb[:], cmpB[0:S, :], AF.Tanh, bias=bcb[:], scale=0.5)
            denr = cp.tile([S, RPC], f32, tag="denr")
            nc.vector.tensor_scalar(denr[:], cmpB[S:128, :], bcd[:], 0.0,
                                    ALU.add, ALU.max)
            nc.vector.tensor_scalar(cmpB[0:S, :], tmpb[:], 0.5, 0.5, ALU.mult, ALU.add)

            # ---- volume rendering composite ----
            tau = cp.tile([S, RPC], f32, tag="tau")
            nc.vector.tensor_mul(tau[:], denr[:], delt[:])
            pep = pl.tile([128, 1024], f32, tag="lp")
            nc.tensor.matmul(pep[0:S, 0:512], tris[:], tau[:], start=True, stop=True)
            inc = cp.tile([S, RPC], f32, tag="inc")
            nc.vector.tensor_add(inc[:], pep[0:S, 0:512], tau[:])
            exc2 = cp.tile([S, RPC], f32, tag="exc2")
            nc.vector.tensor_sub(exc2[:], inc[:], tau[:])
            trans = cp.tile([S, RPC], f32, tag="trans")
            nc.scalar.activation(trans[:], exc2[:], AF.Exp, scale=-1.0)
            ee = cp.tile([S, RPC], f32, tag="ee")
            nc.scalar.activation(ee[:], tau[:], AF.Exp, scale=-1.0)
            alpha = cp.tile([S, RPC], f32, tag="alpha")
            nc.vector.tensor_scalar(alpha[:], ee[:], -1.0, 1.0, ALU.mult, ALU.add)
            wt = cp.tile([S, RPC], f32, tag="wt")
            nc.vector.tensor_mul(wt[:], alpha[:], trans[:])
            w2 = cp.tile([128, RPC], f32, tag="w2")
            nc.sync.dma_start(w2[0:S, :], wt[:])
            nc.sync.dma_start(w2[S:128, :], wt[:])
            nc.sync.dma_start(cmpB[S:128, :], dd[:])
            wa = cp.tile([128, RPC], f32, tag="wa")
            nc.vector.tensor_mul(wa[:], w2[:], cmpA[:])
            wb = cp.tile([128, RPC], f32, tag="wb")
            nc.vector.tensor_mul(wb[:], w2[:], cmpB[:])
            nc.tensor.matmul(pep[0:2, 512:1024], onesb[:], wa[:], start=True,
                             stop=True, tile_position=(0, 0))
            nc.tensor.matmul(pep[32:34, 512:1024], onesb[:], wb[:], start=True,
                             stop=True, tile_position=(0, 32))
            outsb = cp.tile([S, RPC], f32, tag="outsb")
            nc.vector.tensor_copy(outsb[0:2, :], pep[0:2, 512:1024])
            nc.vector.tensor_copy(outsb[32:34, :], pep[32:34, 512:1024])
            nc.sync.dma_start(d_out[0:2, :], outsb[0:2, :])
            nc.sync.dma_start(d_out[2:4, :], outsb[32:34, :])

    nc.compile()
    return nc


def _prep(inputs):
    jt = np.ascontiguousarray(np.asarray(inputs["jitter"], np.float32).T)
    rpt = np.ascontiguousarray(np.asarray(inputs["ray_pos"], np.float32).T)
    rdt = np.ascontiguousarray(np.asarray(inputs["ray_dir"], np.float32).T)
    w_in = np.asarray(inputs["w_in"], np.float32)
    perm = np.empty(ENC, np.int64)
    for r in range(ENC):
        base = 0 if r < 30 else 10
        rr = r % 30
        perm[r] = (rr // 10) * 20 + base + (rr % 10)
    win_p = np.ascontiguousarray(w_in[perm])
    w8 = win_p.astype(E4)
    win8 = np.zeros((124, WIDTH), E4)
    win8[0:60] = w8
    win8[64:124] = w8

    w_hid = np.asarray(inputs["w_hid"], np.float32)
    HB = 7 * 2 * 128
    whid_hi = np.empty((128, 2, HB), E4)
    whid_cat = np.empty((128, 7 * 2 * WIDTH), np.float32)
    for l in range(7):
        W = w_hid[l]  # [256, 256]
        Wh = W.astype(E4)
        for m in range(2):
            blk = (l * 2 + m) * 128
            cols = slice(m * 128, (m + 1) * 128)
            whid_hi[:, 0, blk:blk + 128] = Wh[0:128, cols]
            whid_hi[:, 1, blk:blk + 128] = Wh[128:256, cols]
        for kc in range(2):
            whid_cat[:, (l * 2 + kc) * WIDTH:(l * 2 + kc + 1) * WIDTH] = \
                W[kc * 128:(kc + 1) * 128, :]

    whd = np.concatenate([np.asarray(inputs["w_rgb"], np.float32),
                          np.asarray(inputs["w_den"], np.float32)], axis=1)
    whd_h = whd.astype(E4)
    whd_hi = np.zeros((128, 2, 16), E4)
    whd_hi[:, 0, 0:4] = whd_h[0:128]
    whd_hi[:, 1, 0:4] = whd_h[128:256]
    whd_cat = np.empty((128, 8), np.float32)
    whd_cat[:, 0:4] = whd[0:128]
    whd_cat[:, 4:8] = whd[128:256]

    b_rgb = np.asarray(inputs["b_rgb"], np.float32)
    b_den = np.asarray(inputs["b_den"], np.float32)
    bca = np.zeros((128, 1), np.float32)
    bca[0:S] = 0.5 * b_rgb[0]
    bca[S:128] = 0.5 * b_rgb[1]
    bcb = np.full((S, 1), 0.5 * b_rgb[2], np.float32)
    bcd = np.full((S, 1), b_den[0], np.float32)

    # brep for 2-sample-packed fp32 encoder matmul: rows 32j+i of jpos map to
    # output rows 60*(j&1) + perm-row r with weight 2^k
    brepa01 = np.zeros((128, 124), np.float32)
    brepa23 = np.zeros((128, 124), np.float32)
    for r in range(ENC):
        rr = r % 30
        i, k = rr // 10, rr % 10
        brepa01[0 + i, r] = float(2.0 ** k)
        brepa01[32 + i, 64 + r] = float(2.0 ** k)
        brepa23[64 + i, r] = float(2.0 ** k)
        brepa23[96 + i, 64 + r] = float(2.0 ** k)
    mbias = np.zeros((124, 1), np.float32)
    mbias[30:60] = 0.25
    mbias[94:124] = 0.25
    b2v = np.zeros((124, 1), np.float32)
    b2v[30:60] = np.float32(np.pi / 2)
    b2v[94:124] = np.float32(np.pi / 2)
    # fast-path brep: rows scaled by 1/2pi, plus mbias via the ones-row 32j+3
    inv2pi = np.float64(1.0) / (2 * np.pi)
    brep01 = brepa01 * np.float32(inv2pi)
    brep23 = brepa23 * np.float32(inv2pi)
    for r in range(124):
        if mbias[r, 0]:
            if r < 64:
                brep01[3, r] = mbias[r, 0]
                brep23[67, r] = mbias[r, 0]
            else:
                brep01[35, r] = mbias[r, 0]
                brep23[99, r] = mbias[r, 0]
    # sin bias column: b2 - 2pi*mbias
    colc = (b2v - np.float32(2 * np.pi) * mbias).astype(np.float32)
    iota = np.arange(S, dtype=np.float32).reshape(S, 1)
    tris = (np.arange(S)[:, None] < np.arange(S)[None, :]).astype(np.float32)
    onesb = np.zeros((128, 2), np.float32)
    onesb[:64, 0] = 1.0
    onesb[64:, 1] = 1.0
    big = np.full((1, RPC), 1e10, np.float32)
    common = dict(win8=win8, win_32=win_p,
                  whid_hi=whid_hi, whid_32=whid_cat,
                  whd_hi=whd_hi, whd_32=whd_cat,
                  brep01=brep01, brep23=brep23,
                  brepa01=brepa01, brepa23=brepa23, colc=colc,
                  bca=bca, bcb=bcb, bcd=bcd, mbias=mbias, b2=b2v,
                  iota=iota, tris=tris, onesb=onesb, big=big)
    in_maps = []
    for c in range(N_CORES):
        sl = slice(c * RPC, (c + 1) * RPC)
        m = dict(common)
        m["jitter_t"] = np.ascontiguousarray(jt[:, sl])
        m["ray_pos_t"] = np.ascontiguousarray(rpt[:, sl])
        m["ray_dir_t"] = np.ascontiguousarray(rdt[:, sl])
        in_maps.append(m)
    return in_maps


def kernel(**inputs):
    global LAST_EXEC_NS
    from concourse.bass_utils import run_bass_kernel_spmd
    if "nc" not in _CACHE:
        _CACHE["nc"] = _build_nc()
    nc = _CACHE["nc"]
    in_maps = _prep(inputs)
    trace = bool(os.environ.get("KERNEL_TRACE"))
    res = run_bass_kernel_spmd(nc, in_maps, core_ids=list(range(N_CORES)),
                               trace=trace)
    LAST_EXEC_NS = getattr(res, "exec_time_ns", None)
    _CACHE["last_results"] = res.results
    _CACHE["last_res"] = res
    out = np.empty((N_CORES * RPC, 4), np.float32)
    for c in range(N_CORES):
        out[c * RPC:(c + 1) * RPC] = res.results[c]["out"].T
    return out
